# revision 1
# baseline (speedup 1.0000x reference)
"""AttnBlock (GroupNorm + single-head spatial attention + residual) on 8
Trainium2 NeuronCores.

Sharding: data-parallel over B (4 batches) x 2-way query-sequence parallel =
8 shards. Each core gets the full x[b] (rolled so its query half is the
first 2048 spatial positions), computes GroupNorm + Q/K/V projections +
attention for its 2048 queries + output projection + residual, and writes a
[512, 2048] slice of the output.

Compute layout (per core, C=512, S=4096, Sq=2048):
  x        [c, s]   4 chunks of [128, 4096] f32 (channels on partitions)
  h = GN(x)         4 chunks of [128, 4096] f16
  q = Wq h + bq     [128, 2048] f16 x4 (out-channels on partitions)
  k = Wk h + bk     [128, 4096] f16 x4
  vT = h^T Wv^T+bv  32 tiles of [128, 512] f16 (spatial on partitions!)
  scoresT[s,q] = k^T q   computed per (128-key-tile x 512-query-block) in
                 PSUM, exp()'d on ScalarE into SBUF f16 -- no transposes
                 anywhere: both AV operands already have s on partitions.
  out'[c,q] += vT^T e    accumulated over all 32 key tiles in 4 PSUM banks
  Z[q]     += 1^T e      (ones-matmul row)
  attn = out'/Z, proj = Wo attn + bo, out = x[:, :2048] + proj

GroupNorm stats use bn_stats/bn_aggr per channel + tiny indicator matmuls to
reduce/broadcast across the 16 channels of each group (cross-partition).

All heavy matmuls run in fp16 (1 PE cycle/row vs 4 for fp32); fp32 would be
~4x slower and fp16 end-to-end error is ~1e-4 of absmax (validated against
the fp32 reference).
"""
import numpy as np

import bass_rust
import concourse.bass as bass
import concourse.tile as tile
from concourse import mybir
from concourse.bass_utils import run_bass_kernel_spmd

F32 = mybir.dt.float32
F16 = mybir.dt.float16
AF = mybir.ActivationFunctionType
ALU = mybir.AluOpType

B, C, H, W = 4, 512, 64, 64
S = H * W            # 4096 spatial positions (keys)
SQ = S // 2          # 2048 queries per core
CC = C // 128        # 4 channel chunks
ST = S // 128        # 32 key tiles
QB = SQ // 512       # 4 query blocks
NG = 32              # groups
GS = C // NG         # 16 channels per group
EPS = 1e-6
SCALE = 1.0 / float(np.sqrt(C))


def _split_excess_waits(nc, max_waits=1):
    """walrus in this toolchain rejects instructions with >1 sync-wait.
    Hoist excess waits onto same-engine NOPs placed just before the
    instruction (engine streams are in-order, so this is equivalent)."""
    for f in nc.m.functions:
        for bb in f.blocks:
            out = []
            for inst in bb.instructions:
                si = inst.sync_info
                if si is not None and len(si.on_wait) > max_waits:
                    waits = list(si.on_wait)
                    plain = [w for w in waits if w.wait_reg is None]
                    special = [w for w in waits if w.wait_reg is not None]
                    n_keep = max(0, max_waits - len(special))
                    hoist = plain[: len(plain) - n_keep] if n_keep < len(plain) else []
                    keep = plain[len(hoist):] + special
                    if len(keep) > max_waits:
                        out.append(inst)
                        continue
                    for j, w in enumerate(hoist):
                        nop = mybir.InstNoOp(name=f"{inst.name}-wsplit{j}")
                        nop.engine = inst.engine
                        nop.sync_info = bass_rust.SyncInfo(on_wait=[w], on_update=[])
                        out.append(nop)
                    inst.sync_info = bass_rust.SyncInfo(
                        on_wait=keep, on_update=list(si.on_update))
                out.append(inst)
            bb.instructions = out


def _build():
    nc = bass.Bass(trn_type="TRN2")

    x_d = nc.dram_tensor("x", [C, S], F32, kind="ExternalInput")
    w_d = {n: nc.dram_tensor(n, [C, C], F16, kind="ExternalInput")
           for n in ("wqT", "wkT", "wvT", "woT")}
    bq_d = nc.dram_tensor("bqc", [128, CC], F32, kind="ExternalInput")
    bk_d = nc.dram_tensor("bkc", [128, CC], F32, kind="ExternalInput")
    bo_d = nc.dram_tensor("boc", [128, CC], F32, kind="ExternalInput")
    bv_d = nc.dram_tensor("bv16", [1, C], F16, kind="ExternalInput")
    ga_d = nc.dram_tensor("gammac", [128, CC], F32, kind="ExternalInput")
    be_d = nc.dram_tensor("betac", [128, CC], F32, kind="ExternalInput")
    ind_d = nc.dram_tensor("ind", [128, CC, NG], F32, kind="ExternalInput")
    indT_d = nc.dram_tensor("indT", [NG, CC, 128], F32, kind="ExternalInput")
    out_d = nc.dram_tensor("out", [CC, 128, SQ], F32, kind="ExternalOutput")

    with tile.TileContext(nc) as tc:
        from contextlib import ExitStack
        with ExitStack() as stack:
            const = stack.enter_context(tc.tile_pool(name="const", bufs=1))
            work = stack.enter_context(tc.tile_pool(name="work", bufs=3))
            p_res = stack.enter_context(tc.tile_pool(name="p_res", bufs=1))
            p_h = stack.enter_context(tc.tile_pool(name="p_h", bufs=1))

            # ---- constants ----
            w_sb = {}
            for n in ("wqT", "wkT", "wvT", "woT"):
                t = const.tile([128, CC, C], F16, name=f"{n}_sb")
                nc.sync.dma_start(out=t[:], in_=w_d[n].rearrange(
                    "(c p) o -> p c o", p=128))
                w_sb[n] = t
            bq_sb = const.tile([128, CC], F32, name="bq_sb")
            nc.sync.dma_start(out=bq_sb[:], in_=bq_d[:, :])
            bk_sb = const.tile([128, CC], F32, name="bk_sb")
            nc.sync.dma_start(out=bk_sb[:], in_=bk_d[:, :])
            bo_sb = const.tile([128, CC], F32, name="bo_sb")
            nc.sync.dma_start(out=bo_sb[:], in_=bo_d[:, :])
            bv_sb = const.tile([1, C], F16, name="bv_sb")
            nc.sync.dma_start(out=bv_sb[:], in_=bv_d[:, :])
            ga_sb = const.tile([128, CC], F32, name="ga_sb")
            nc.sync.dma_start(out=ga_sb[:], in_=ga_d[:, :])
            be_sb = const.tile([128, CC], F32, name="be_sb")
            nc.sync.dma_start(out=be_sb[:], in_=be_d[:, :])
            ind_sb = const.tile([128, CC, NG], F32, name="ind_sb")
            nc.sync.dma_start(out=ind_sb[:], in_=ind_d[:, :, :])
            indT_sb = const.tile([NG, CC, 128], F32, name="indT_sb")
            nc.sync.dma_start(out=indT_sb[:], in_=indT_d[:, :, :])

            ones_r16 = const.tile([1, 128], F16, name="ones_r16")
            nc.vector.memset(ones_r16[:], 1.0)
            ones_c16 = const.tile([128, 1], F16, name="ones_c16")
            nc.vector.memset(ones_c16[:], 1.0)
            ones_r32 = const.tile([1, 128], F32, name="ones_r32")
            nc.vector.memset(ones_r32[:], 1.0)
            eps_sb = const.tile([NG, 1], F32, name="eps_sb")
            nc.vector.memset(eps_sb[:], EPS)

            h16 = p_h.tile([128, CC, S], F16, name="h16")
            xres = p_res.tile([128, CC, SQ], F32, name="xres")

            # =========== Phase 1: load x + GroupNorm ===========
            with tc.tile_pool(name="p_x", bufs=1) as p_x, \
                 tc.tile_pool(name="ps_gn", bufs=2, space="PSUM") as ps_gn:
                xc = p_x.tile([128, CC, S], F32, name="xc")
                for i in range(CC):
                    nc.sync.dma_start(
                        out=xc[:, i, :],
                        in_=x_d[i * 128:(i + 1) * 128, :])

                # copy residual half out before x dies
                for i in range(CC):
                    nc.vector.tensor_copy(xres[:, i, :], xc[:, i, :SQ])

                # per-channel mean/var via bn_stats/bn_aggr (free-dim)
                FM = nc.vector.BN_STATS_FMAX
                nsub = S // FM
                stats2 = []
                for i in range(CC):
                    st_t = work.tile([128, nsub, nc.vector.BN_STATS_DIM], F32,
                                     name="st_t", tag="gn_st", bufs=4)
                    xi = xc[:, i, :].rearrange("p (n f) -> p n f", f=FM)
                    for j in range(nsub):
                        nc.vector.bn_stats(out=st_t[:, j, :], in_=xi[:, j, :])
                    mv = work.tile([128, nc.vector.BN_AGGR_DIM], F32,
                                   name="mv", tag="gn_mv", bufs=4)
                    nc.vector.bn_aggr(out=mv[:], in_=st_t[:])
                    # stats2 = [mean, E[x^2]] per channel
                    s2 = work.tile([128, 2], F32, name="s2", tag="gn_s2", bufs=4)
                    nc.vector.tensor_copy(s2[:, 0:1], mv[:, 0:1])
                    msq = work.tile([128, 1], F32, name="msq", tag="gn_msq", bufs=4)
                    nc.vector.tensor_mul(msq[:], mv[:, 0:1], mv[:, 0:1])
                    nc.vector.tensor_add(s2[:, 1:2], mv[:, 1:2], msq[:])
                    stats2.append(s2)

                # reduce over the 16 channels of each group: indicator matmul
                psg = ps_gn.tile([NG, 2], F32, name="psg")
                for i in range(CC):
                    nc.tensor.matmul(psg[:], ind_sb[:, i, :], stats2[i][:],
                                     start=(i == 0), stop=(i == CC - 1))
                gstat = work.tile([NG, 2], F32, name="gstat")
                nc.scalar.mul(gstat[:], psg[:], 1.0 / GS)  # [mean_g, E_g[x^2]]

                # var_g, rstd_g (+ one Newton step to clean up ACT sqrt)
                mg2 = work.tile([NG, 1], F32, name="mg2")
                nc.vector.tensor_mul(mg2[:], gstat[:, 0:1], gstat[:, 0:1])
                varg = work.tile([NG, 1], F32, name="varg")
                nc.vector.tensor_tensor(out=varg[:], in0=gstat[:, 1:2],
                                        in1=mg2[:], op=ALU.subtract)
                sd = work.tile([NG, 1], F32, name="sd")
                nc.scalar.activation(sd[:], varg[:], AF.Sqrt, bias=eps_sb[:])
                r0 = work.tile([NG, 1], F32, name="r0")
                nc.vector.reciprocal(r0[:], sd[:])
                ve = work.tile([NG, 1], F32, name="ve")
                nc.vector.tensor_scalar_add(ve[:], varg[:], EPS)
                r0sq = work.tile([NG, 1], F32, name="r0sq")
                nc.vector.tensor_mul(r0sq[:], r0[:], r0[:])
                t2 = work.tile([NG, 1], F32, name="t2")
                nc.vector.tensor_mul(t2[:], ve[:], r0sq[:])
                t3 = work.tile([NG, 1], F32, name="t3")
                nc.vector.tensor_scalar(out=t3[:], in0=t2[:], scalar1=-0.5,
                                        scalar2=1.5, op0=ALU.mult, op1=ALU.add)
                gv = work.tile([NG, 2], F32, name="gv")  # [mean_g, rstd_g]
                nc.vector.tensor_copy(gv[:, 0:1], gstat[:, 0:1])
                nc.vector.tensor_mul(gv[:, 1:2], r0[:], t3[:])

                # broadcast group stats back to channels, fold gamma/beta
                for i in range(CC):
                    psb = ps_gn.tile([128, 2], F32, name="psb")
                    nc.tensor.matmul(psb[:], indT_sb[:, i, :], gv[:],
                                     start=True, stop=True)
                    mr = work.tile([128, 2], F32, name="mr", tag="gn_mr", bufs=4)
                    nc.scalar.copy(mr[:], psb[:])
                    sc_c = work.tile([128, 1], F32, name="sc_c", tag="gn_sc", bufs=4)
                    nc.vector.tensor_mul(sc_c[:], mr[:, 1:2], ga_sb[:, i:i + 1])
                    mt = work.tile([128, 1], F32, name="mt", tag="gn_mt", bufs=4)
                    nc.vector.tensor_mul(mt[:], mr[:, 0:1], sc_c[:])
                    bi_c = work.tile([128, 1], F32, name="bi_c", tag="gn_bi", bufs=4)
                    nc.vector.tensor_tensor(out=bi_c[:], in0=be_sb[:, i:i + 1],
                                            in1=mt[:], op=ALU.subtract)
                    # h = x*scale + bias, cast to f16
                    nc.vector.tensor_scalar(
                        out=h16[:, i, :], in0=xc[:, i, :],
                        scalar1=sc_c[:], scalar2=bi_c[:],
                        op0=ALU.mult, op1=ALU.add)

            # =========== Phase 2: projections ===========
            p_kv = stack.enter_context(tc.tile_pool(name="p_kv", bufs=1))
            k16 = p_kv.tile([128, CC, S], F16, name="k16")
            q16 = p_kv.tile([128, CC, SQ], F16, name="q16")
            vT16 = p_kv.tile([128, ST, C], F16, name="vT16")

            with tc.tile_pool(name="ps_proj", bufs=3, space="PSUM") as ps_p:
                # q = WqT^T h (+bq): only the first SQ columns of h
                for oc in range(CC):
                    for qb in range(SQ // 512):
                        pt = ps_p.tile([128, 512], F32, name="pt", tag="pp")
                        cols = slice(qb * 512, (qb + 1) * 512)
                        for ic in range(CC):
                            nc.tensor.matmul(
                                pt[:], w_sb["wqT"][:, ic, oc * 128:(oc + 1) * 128],
                                h16[:, ic, cols],
                                start=(ic == 0), stop=(ic == CC - 1))
                        nc.scalar.activation(q16[:, oc, cols], pt[:],
                                             AF.Identity, bias=bq_sb[:, oc:oc + 1])
                # k = WkT^T h (+bk): all S columns
                for oc in range(CC):
                    for sb in range(S // 512):
                        pt = ps_p.tile([128, 512], F32, name="pt", tag="pp")
                        cols = slice(sb * 512, (sb + 1) * 512)
                        for ic in range(CC):
                            nc.tensor.matmul(
                                pt[:], w_sb["wkT"][:, ic, oc * 128:(oc + 1) * 128],
                                h16[:, ic, cols],
                                start=(ic == 0), stop=(ic == CC - 1))
                        nc.scalar.activation(k16[:, oc, cols], pt[:],
                                             AF.Identity, bias=bk_sb[:, oc:oc + 1])
                # vT[s, c] = h[:, s]^T WvT (+bv broadcast via ones-matmul)
                for st in range(ST):
                    pt = ps_p.tile([128, 512], F32, name="pt", tag="pp")
                    scols = slice(st * 128, (st + 1) * 128)
                    for ic in range(CC):
                        nc.tensor.matmul(pt[:], h16[:, ic, scols],
                                         w_sb["wvT"][:, ic, :],
                                         start=(ic == 0), stop=False)
                    nc.tensor.matmul(pt[:], ones_r16[:], bv_sb[:],
                                     start=False, stop=True)
                    nc.scalar.copy(vT16[:, st, :], pt[:])

            # =========== Phase 3: attention + out-projection ===========
            with tc.tile_pool(name="ps_po", bufs=4, space="PSUM") as ps_po, \
                 tc.tile_pool(name="ps_z", bufs=2, space="PSUM") as ps_z, \
                 tc.tile_pool(name="ps_s", bufs=2, space="PSUM") as ps_s:
                for qb in range(QB):
                    qcols = slice(qb * 512, (qb + 1) * 512)
                    po = [ps_po.tile([128, 512], F32, name="po", tag="po")
                          for _ in range(CC)]
                    pz = ps_z.tile([1, 512], F32, name="pz", tag="pz")
                    for st in range(ST):
                        pscore = ps_s.tile([128, 512], F32, name="pscore",
                                           tag="msum")
                        scols = slice(st * 128, (st + 1) * 128)
                        for ic in range(CC):
                            nc.tensor.matmul(pscore[:], k16[:, ic, scols],
                                             q16[:, ic, qcols],
                                             start=(ic == 0), stop=(ic == CC - 1))
                        e16 = work.tile([128, 512], F16, name="e16",
                                        tag="e16", bufs=3)
                        nc.scalar.activation(e16[:], pscore[:], AF.Exp,
                                             scale=SCALE)
                        for cc2 in range(CC):
                            nc.tensor.matmul(
                                po[cc2][:],
                                vT16[:, st, cc2 * 128:(cc2 + 1) * 128],
                                e16[:],
                                start=(st == 0), stop=(st == ST - 1))
                        nc.tensor.matmul(pz[:], ones_c16[:], e16[:],
                                         start=(st == 0), stop=(st == ST - 1))
                    # normalize: attn = po / Z
                    z_sb = work.tile([1, 512], F32, name="z_sb", tag="z_sb")
                    nc.scalar.copy(z_sb[:], pz[:])
                    rz = work.tile([1, 512], F32, name="rz", tag="rz")
                    nc.vector.reciprocal(rz[:], z_sb[:])
                    pzb = ps_z.tile([128, 512], F32, name="pzb", tag="pz")
                    nc.tensor.matmul(pzb[:], ones_r32[:], rz[:],
                                     start=True, stop=True)
                    rzb = work.tile([128, 512], F32, name="rzb", tag="rzb",
                                    bufs=2)
                    nc.scalar.copy(rzb[:], pzb[:])
                    attn = [work.tile([128, 512], F16, name="attn",
                                      tag="attn", bufs=8) for _ in range(CC)]
                    for cc2 in range(CC):
                        nc.vector.tensor_mul(attn[cc2][:], po[cc2][:], rzb[:])
                    # out-projection + bias + residual + store
                    for oc in range(CC):
                        pp = ps_s.tile([128, 512], F32, name="pp", tag="msum")
                        for cc2 in range(CC):
                            nc.tensor.matmul(
                                pp[:], w_sb["woT"][:, cc2, oc * 128:(oc + 1) * 128],
                                attn[cc2][:],
                                start=(cc2 == 0), stop=(cc2 == CC - 1))
                        p32 = work.tile([128, 512], F32, name="p32", tag="p32")
                        nc.scalar.activation(p32[:], pp[:], AF.Identity,
                                             bias=bo_sb[:, oc:oc + 1])
                        o32 = work.tile([128, 512], F32, name="o32", tag="o32")
                        nc.vector.tensor_add(o32[:], p32[:], xres[:, oc, qcols])
                        nc.sync.dma_start(out=out_d[oc, :, qcols], in_=o32[:])

    _split_excess_waits(nc)
    return nc


_cache = {}


def _get_program():
    if "nc" not in _cache:
        _cache["nc"] = _build()
    return _cache["nc"]


def kernel(x, gamma, beta, wq, bq, wk, bk, wv, bv, wo, bo, trace=False):
    x = np.asarray(x, dtype=np.float32)
    gamma = np.asarray(gamma, dtype=np.float32)
    beta = np.asarray(beta, dtype=np.float32)
    wq, wk, wv, wo = (np.asarray(a, dtype=np.float32) for a in (wq, wk, wv, wo))
    bq, bk, bv, bo = (np.asarray(a, dtype=np.float32) for a in (bq, bk, bv, bo))

    nc = _get_program()

    shared = {
        "wqT": np.ascontiguousarray(wq.T).astype(np.float16),
        "wkT": np.ascontiguousarray(wk.T).astype(np.float16),
        "wvT": np.ascontiguousarray(wv.T).astype(np.float16),
        "woT": np.ascontiguousarray(wo.T).astype(np.float16),
        "bqc": np.ascontiguousarray(bq.reshape(CC, 128).T),
        "bkc": np.ascontiguousarray(bk.reshape(CC, 128).T),
        "boc": np.ascontiguousarray(bo.reshape(CC, 128).T),
        "bv16": bv.reshape(1, C).astype(np.float16),
        "gammac": np.ascontiguousarray(gamma.reshape(CC, 128).T),
        "betac": np.ascontiguousarray(beta.reshape(CC, 128).T),
    }
    ind = np.zeros((128, CC, NG), np.float32)
    indT = np.zeros((NG, CC, 128), np.float32)
    for i in range(CC):
        for p in range(128):
            g = (i * 128 + p) // GS
            ind[p, i, g] = 1.0
            indT[g, i, p] = 1.0
    shared["ind"] = ind
    shared["indT"] = indT

    in_maps = []
    for core in range(8):
        b, half = core // 2, core % 2
        xs = x[b].reshape(C, S)
        if half:
            xin = np.concatenate([xs[:, SQ:], xs[:, :SQ]], axis=1)
        else:
            xin = np.ascontiguousarray(xs)
        in_maps.append({"x": xin, **shared})

    res = run_bass_kernel_spmd(nc, in_maps, core_ids=list(range(8)),
                               trace=trace)
    _cache["last_exec_time_ns"] = res.exec_time_ns

    y = np.empty((B, C, S), np.float32)
    for core in range(8):
        b, half = core // 2, core % 2
        y[b, :, half * SQ:(half + 1) * SQ] = \
            res.results[core]["out"].reshape(C, SQ)
    return y.reshape(B, C, H, W)


# revision 15
# speedup vs baseline: 1.0488x; 1.0488x over previous
"""AttnBlock (GroupNorm + single-head spatial attention + residual) on 8
Trainium2 NeuronCores.

Sharding: data-parallel over B (4 batches) x 2-way query-sequence parallel =
8 shards. Each core gets the full x[b] (rolled so its query half is the
first 2048 spatial positions), computes GroupNorm + Q/K/V projections +
attention for its 2048 queries + output projection + residual, and writes a
[512, 2048] slice of the output.

Compute layout (per core, C=512, S=4096, Sq=2048):
  x        [c, s]   4 chunks of [128, 4096] f32 (channels on partitions)
  h = GN(x)         4 chunks of [128, 4096] f16
  q = Wq h + bq     [128, 2048] f16 x4 (out-channels on partitions)
  k = Wk h + bk     [128, 4096] f16 x4
  vT = h^T Wv^T+bv  32 tiles of [128, 512] f16 (spatial on partitions!)
  scoresT[s,q] = k^T q   computed per (128-key-tile x 512-query-block) in
                 PSUM, exp()'d on ScalarE into SBUF f16 -- no transposes
                 anywhere: both AV operands already have s on partitions.
  out'[c,q] += vT^T e    accumulated over all 32 key tiles in 4 PSUM banks
  Z[q]     += 1^T e      (ones-matmul row)
  attn = out'/Z, proj = Wo attn + bo, out = x[:, :2048] + proj

GroupNorm stats use bn_stats/bn_aggr per channel + tiny indicator matmuls to
reduce/broadcast across the 16 channels of each group (cross-partition).

All heavy matmuls run in fp16 (1 PE cycle/row vs 4 for fp32); fp32 would be
~4x slower and fp16 end-to-end error is ~1e-4 of absmax (validated against
the fp32 reference).
"""
import numpy as np

import bass_rust
import concourse.bass as bass
import concourse.tile as tile
from concourse import mybir
from concourse.bass_utils import run_bass_kernel_spmd

F32 = mybir.dt.float32
F32R = mybir.dt.float32r
F16 = mybir.dt.float16
AF = mybir.ActivationFunctionType
ALU = mybir.AluOpType

B, C, H, W = 4, 512, 64, 64
S = H * W            # 4096 spatial positions (keys)
SQ = S // 2          # 2048 queries per core
CC = C // 128        # 4 channel chunks
ST = S // 128        # 32 key tiles
QB = SQ // 512       # 4 query blocks
NG = 32              # groups
GS = C // NG         # 16 channels per group
EPS = 1e-6
SCALE = 1.0 / float(np.sqrt(C))
VSCALE = 2.0 ** -6   # pre-scale on v and the Z-ones so |att| stays in f16


def _split_excess_waits(nc, max_waits=1):
    """walrus in this toolchain rejects instructions with >1 sync-wait.
    Hoist excess waits onto same-engine NOPs placed just before the
    instruction (engine streams are in-order, so this is equivalent)."""
    for f in nc.m.functions:
        for bb in f.blocks:
            out = []
            for inst in bb.instructions:
                si = inst.sync_info
                if si is not None and len(si.on_wait) > max_waits:
                    waits = list(si.on_wait)
                    plain = [w for w in waits if w.wait_reg is None]
                    special = [w for w in waits if w.wait_reg is not None]
                    n_keep = max(0, max_waits - len(special))
                    hoist = plain[: len(plain) - n_keep] if n_keep < len(plain) else []
                    keep = plain[len(hoist):] + special
                    if len(keep) > max_waits:
                        out.append(inst)
                        continue
                    for j, w in enumerate(hoist):
                        nop = mybir.InstNoOp(name=f"{inst.name}-wsplit{j}")
                        nop.engine = inst.engine
                        nop.sync_info = bass_rust.SyncInfo(on_wait=[w], on_update=[])
                        out.append(nop)
                    inst.sync_info = bass_rust.SyncInfo(
                        on_wait=keep, on_update=list(si.on_update))
                out.append(inst)
            bb.instructions = out


def _build(with_bv=True):
    nc = bass.Bass(trn_type="TRN2")

    x_d = nc.dram_tensor("x", [C, S], F32, kind="ExternalInput")
    w_d = {n: nc.dram_tensor(n, [C, C], F16, kind="ExternalInput")
           for n in ("wqT", "wkT", "wvT", "woT")}
    bq_d = nc.dram_tensor("bqc", [128, CC], F32, kind="ExternalInput")
    bk_d = nc.dram_tensor("bkc", [128, CC], F32, kind="ExternalInput")
    bo_d = nc.dram_tensor("boc", [128, CC], F32, kind="ExternalInput")
    bv_d = nc.dram_tensor("bv16", [1, C], F16, kind="ExternalInput")
    ga_d = nc.dram_tensor("gammac", [128, CC], F32, kind="ExternalInput")
    be_d = nc.dram_tensor("betac", [128, CC], F32, kind="ExternalInput")
    ind_d = nc.dram_tensor("ind", [128, CC, NG], F32, kind="ExternalInput")
    indT_d = nc.dram_tensor("indT", [NG, CC, 128], F32, kind="ExternalInput")
    out_d = nc.dram_tensor("out", [CC, 128, SQ], F32, kind="ExternalOutput")

    with tile.TileContext(nc) as tc:
        from contextlib import ExitStack
        with ExitStack() as stack:
            const = stack.enter_context(tc.tile_pool(name="const", bufs=1))
            work = stack.enter_context(tc.tile_pool(name="work", bufs=3))
            p_res = stack.enter_context(tc.tile_pool(name="p_res", bufs=1))
            p_h = stack.enter_context(tc.tile_pool(name="p_h", bufs=1))

            # ---- constants ----
            w_sb = {}
            for n in ("wqT", "wkT", "wvT", "woT"):
                t = const.tile([128, CC, C], F16, name=f"{n}_sb")
                nc.sync.dma_start(out=t[:], in_=w_d[n].rearrange(
                    "(c p) o -> p c o", p=128))
                w_sb[n] = t
            bq_sb = const.tile([128, CC], F32, name="bq_sb")
            nc.sync.dma_start(out=bq_sb[:], in_=bq_d[:, :])
            bk_sb = const.tile([128, CC], F32, name="bk_sb")
            nc.sync.dma_start(out=bk_sb[:], in_=bk_d[:, :])
            bo_sb = const.tile([128, CC], F32, name="bo_sb")
            nc.sync.dma_start(out=bo_sb[:], in_=bo_d[:, :])
            bv_sb = const.tile([1, C], F16, name="bv_sb")
            nc.sync.dma_start(out=bv_sb[:], in_=bv_d[:, :])
            ga_sb = const.tile([128, CC], F32, name="ga_sb")
            nc.sync.dma_start(out=ga_sb[:], in_=ga_d[:, :])
            be_sb = const.tile([128, CC], F32, name="be_sb")
            nc.sync.dma_start(out=be_sb[:], in_=be_d[:, :])
            ind_sb = const.tile([128, CC, NG], F32, name="ind_sb")
            nc.sync.dma_start(out=ind_sb[:], in_=ind_d[:, :, :])
            indT_sb = const.tile([NG, CC, 128], F32, name="indT_sb")
            nc.sync.dma_start(out=indT_sb[:], in_=indT_d[:, :, :])

            ones_r16 = const.tile([1, 128], F16, name="ones_r16")
            nc.vector.memset(ones_r16[:], 1.0)
            ones_c16 = const.tile([128, 1], F16, name="ones_c16")
            nc.vector.memset(ones_c16[:], VSCALE)
            ones_r32 = const.tile([1, 128], F32R, name="ones_r32")
            nc.vector.memset(ones_r32[:].bitcast(F32), 1.0)
            eps_sb = const.tile([NG, 1], F32, name="eps_sb")
            nc.vector.memset(eps_sb[:], EPS)

            h16 = p_h.tile([128, CC, S], F16, name="h16")
            xres = p_res.tile([128, CC, SQ], F32, name="xres")

            # =========== Phase 1: load x + GroupNorm ===========
            with tc.tile_pool(name="p_x", bufs=1) as p_x, \
                 tc.tile_pool(name="ps_gn", bufs=2, space="PSUM") as ps_gn:
                xc = p_x.tile([128, CC, S], F32, name="xc")
                for i in range(CC):
                    for hh in range(2):
                        cols = slice(hh * SQ, (hh + 1) * SQ)
                        nc.sync.dma_start(
                            out=xc[:, i, cols],
                            in_=x_d[i * 128:(i + 1) * 128, cols])

                # copy residual half out before x dies
                for i in range(CC):
                    nc.vector.tensor_copy(xres[:, i, :], xc[:, i, :SQ])

                # per-channel mean/var via bn_stats/bn_aggr (free-dim)
                FM = nc.vector.BN_STATS_FMAX
                nsub = S // FM
                stats2 = []
                for i in range(CC):
                    st_t = work.tile([128, nsub, nc.vector.BN_STATS_DIM], F32,
                                     name="st_t", tag="gn_st", bufs=4)
                    xi = xc[:, i, :].rearrange("p (n f) -> p n f", f=FM)
                    for j in range(nsub):
                        nc.vector.bn_stats(out=st_t[:, j, :], in_=xi[:, j, :])
                    mv = work.tile([128, nc.vector.BN_AGGR_DIM], F32,
                                   name="mv", tag="gn_mv", bufs=4)
                    nc.vector.bn_aggr(out=mv[:], in_=st_t[:])
                    # stats2 = [mean, E[x^2]] per channel
                    s2 = work.tile([128, 2], F32, name="s2", tag="gn_s2", bufs=4)
                    nc.vector.tensor_copy(s2[:, 0:1], mv[:, 0:1])
                    msq = work.tile([128, 1], F32, name="msq", tag="gn_msq", bufs=4)
                    nc.vector.tensor_mul(msq[:], mv[:, 0:1], mv[:, 0:1])
                    nc.vector.tensor_add(s2[:, 1:2], mv[:, 1:2], msq[:])
                    stats2.append(s2)

                # reduce over the 16 channels of each group: indicator matmul
                psg = ps_gn.tile([NG, 2], F32, name="psg")
                for i in range(CC):
                    nc.tensor.matmul(psg[:], ind_sb[:, i, :], stats2[i][:],
                                     start=(i == 0), stop=(i == CC - 1))
                gstat = work.tile([NG, 2], F32, name="gstat")
                nc.scalar.mul(gstat[:], psg[:], 1.0 / GS)  # [mean_g, E_g[x^2]]

                # var_g, rstd_g (+ one Newton step to clean up ACT sqrt)
                mg2 = work.tile([NG, 1], F32, name="mg2")
                nc.vector.tensor_mul(mg2[:], gstat[:, 0:1], gstat[:, 0:1])
                varg = work.tile([NG, 1], F32, name="varg")
                nc.vector.tensor_tensor(out=varg[:], in0=gstat[:, 1:2],
                                        in1=mg2[:], op=ALU.subtract)
                sd = work.tile([NG, 1], F32, name="sd")
                nc.scalar.activation(sd[:], varg[:], AF.Sqrt, bias=eps_sb[:])
                r0 = work.tile([NG, 1], F32, name="r0")
                nc.vector.reciprocal(r0[:], sd[:])
                ve = work.tile([NG, 1], F32, name="ve")
                nc.vector.tensor_scalar_add(ve[:], varg[:], EPS)
                r0sq = work.tile([NG, 1], F32, name="r0sq")
                nc.vector.tensor_mul(r0sq[:], r0[:], r0[:])
                t2 = work.tile([NG, 1], F32, name="t2")
                nc.vector.tensor_mul(t2[:], ve[:], r0sq[:])
                t3 = work.tile([NG, 1], F32, name="t3")
                nc.vector.tensor_scalar(out=t3[:], in0=t2[:], scalar1=-0.5,
                                        scalar2=1.5, op0=ALU.mult, op1=ALU.add)
                gv = work.tile([NG, 2], F32, name="gv")  # [mean_g, rstd_g]
                nc.vector.tensor_copy(gv[:, 0:1], gstat[:, 0:1])
                nc.vector.tensor_mul(gv[:, 1:2], r0[:], t3[:])

                # broadcast group stats back to channels, fold gamma/beta
                for i in range(CC):
                    psb = ps_gn.tile([128, 2], F32, name="psb")
                    nc.tensor.matmul(psb[:], indT_sb[:, i, :], gv[:],
                                     start=True, stop=True)
                    mr = work.tile([128, 2], F32, name="mr", tag="gn_mr", bufs=4)
                    nc.scalar.copy(mr[:], psb[:])
                    sc_c = work.tile([128, 1], F32, name="sc_c", tag="gn_sc", bufs=4)
                    nc.vector.tensor_mul(sc_c[:], mr[:, 1:2], ga_sb[:, i:i + 1])
                    mt = work.tile([128, 1], F32, name="mt", tag="gn_mt", bufs=4)
                    nc.vector.tensor_mul(mt[:], mr[:, 0:1], sc_c[:])
                    bi_c = work.tile([128, 1], F32, name="bi_c", tag="gn_bi", bufs=4)
                    nc.vector.tensor_tensor(out=bi_c[:], in0=be_sb[:, i:i + 1],
                                            in1=mt[:], op=ALU.subtract)
                    # h = x*scale + bias, cast to f16
                    nc.vector.tensor_scalar(
                        out=h16[:, i, :], in0=xc[:, i, :],
                        scalar1=sc_c[:], scalar2=bi_c[:],
                        op0=ALU.mult, op1=ALU.add)

            # =========== Phase 2: projections ===========
            p_kv = stack.enter_context(tc.tile_pool(name="p_kv", bufs=1))
            k16 = p_kv.tile([128, CC, S], F16, name="k16")
            q16 = p_kv.tile([128, CC, SQ], F16, name="q16")
            vT16 = p_kv.tile([128, ST, C], F16, name="vT16")

            with tc.tile_pool(name="ps_proj", bufs=3, space="PSUM") as ps_p:
                # q = WqT^T h (+bq): only the first SQ columns of h
                for oc in range(CC):
                    for qb in range(SQ // 512):
                        pt = ps_p.tile([128, 512], F32, name="pt", tag="pp")
                        cols = slice(qb * 512, (qb + 1) * 512)
                        for ic in range(CC):
                            nc.tensor.matmul(
                                pt[:], w_sb["wqT"][:, ic, oc * 128:(oc + 1) * 128],
                                h16[:, ic, cols],
                                start=(ic == 0), stop=(ic == CC - 1))
                        nc.scalar.activation(q16[:, oc, cols], pt[:],
                                             AF.Identity, bias=bq_sb[:, oc:oc + 1])
                # k = WkT^T h (+bk): all S columns
                for oc in range(CC):
                    for sb in range(S // 512):
                        pt = ps_p.tile([128, 512], F32, name="pt", tag="pp")
                        cols = slice(sb * 512, (sb + 1) * 512)
                        for ic in range(CC):
                            nc.tensor.matmul(
                                pt[:], w_sb["wkT"][:, ic, oc * 128:(oc + 1) * 128],
                                h16[:, ic, cols],
                                start=(ic == 0), stop=(ic == CC - 1))
                        nc.scalar.activation(k16[:, oc, cols], pt[:],
                                             AF.Identity, bias=bk_sb[:, oc:oc + 1])
                # vT[s, c] = h[:, s]^T WvT (+bv broadcast via ones-matmul).
                # vT is stored pre-scaled by 2^-6 (and the Z-ones column uses
                # the same scale) so the unnormalized attention accumulator
                # stays comfortably inside f16 range; the scale cancels in
                # the final (Wo att)/Z normalization.
                for st in range(ST):
                    pt = ps_p.tile([128, 512], F32, name="pt", tag="pp")
                    scols = slice(st * 128, (st + 1) * 128)
                    for ic in range(CC):
                        nc.tensor.matmul(pt[:], h16[:, ic, scols],
                                         w_sb["wvT"][:, ic, :],
                                         start=(ic == 0),
                                         stop=(ic == CC - 1 and not with_bv))
                    if with_bv:
                        nc.tensor.matmul(pt[:], ones_r16[:], bv_sb[:],
                                         start=False, stop=True)
                    nc.scalar.mul(vT16[:, st, :], pt[:], VSCALE)

            # =========== Phase 3: attention + out-projection ===========
            # att (= 2^-6 * sum_s e[s,q] v[:,s], unnormalized) is evacuated
            # to f16 right after the key loop; normalization by 1/Z happens
            # AFTER the out-projection (it commutes with Wo), so the
            # reciprocal/broadcast chain runs on DVE off the PE critical
            # path. The out-projection for block qb is emitted after block
            # qb+1's key loop so its PSUM->f16 dependency is fully hidden.
            with tc.tile_pool(name="ps_po", bufs=4, space="PSUM") as ps_po, \
                 tc.tile_pool(name="ps_z", bufs=2, space="PSUM") as ps_z, \
                 tc.tile_pool(name="ps_s", bufs=2, space="PSUM") as ps_s:

                def emit_outproj(qb, att, rzb):
                    qcols = slice(qb * 512, (qb + 1) * 512)
                    for oc in range(CC):
                        pp = ps_s.tile([128, 512], F32, name="pp", tag="msum")
                        for cc2 in range(CC):
                            nc.tensor.matmul(
                                pp[:],
                                w_sb["woT"][:, cc2, oc * 128:(oc + 1) * 128],
                                att[cc2][:],
                                start=(cc2 == 0), stop=(cc2 == CC - 1))
                        t32 = work.tile([128, 512], F32, name="t32", tag="t32", bufs=2)
                        nc.vector.tensor_mul(t32[:], pp[:], rzb[:])
                        o32 = work.tile([128, 512], F32, name="o32", tag="o32", bufs=2)
                        nc.vector.scalar_tensor_tensor(
                            out=o32[:], in0=t32[:], scalar=bo_sb[:, oc:oc + 1],
                            in1=xres[:, oc, qcols], op0=ALU.add, op1=ALU.add)
                        nc.sync.dma_start(out=out_d[oc, :, qcols], in_=o32[:])

                prev = None
                for qb in range(QB):
                    qcols = slice(qb * 512, (qb + 1) * 512)
                    po = [ps_po.tile([128, 512], F32, name="po", tag="po")
                          for _ in range(CC)]
                    pz = ps_z.tile([1, 512], F32, name="pz", tag="pz")
                    for st in range(ST):
                        pscore = ps_s.tile([128, 512], F32, name="pscore",
                                           tag="msum")
                        scols = slice(st * 128, (st + 1) * 128)
                        for ic in range(CC):
                            nc.tensor.matmul(pscore[:], k16[:, ic, scols],
                                             q16[:, ic, qcols],
                                             start=(ic == 0), stop=(ic == CC - 1))
                        e16 = work.tile([128, 512], F16, name="e16",
                                        tag="e16", bufs=3)
                        nc.scalar.activation(e16[:], pscore[:], AF.Exp,
                                             scale=SCALE)
                        for cc2 in range(CC):
                            nc.tensor.matmul(
                                po[cc2][:],
                                vT16[:, st, cc2 * 128:(cc2 + 1) * 128],
                                e16[:],
                                start=(st == 0), stop=(st == ST - 1))
                        nc.tensor.matmul(pz[:], ones_c16[:], e16[:],
                                         start=(st == 0), stop=(st == ST - 1))
                    # evacuate att to f16 (frees po for the next block) and
                    # compute 1/Z broadcast, all on DVE
                    att = [work.tile([128, 512], F16, name="att",
                                     tag="att", bufs=8) for _ in range(CC)]
                    for cc2 in range(CC):
                        nc.vector.tensor_copy(att[cc2][:], po[cc2][:])
                    z_sb = work.tile([1, 512], F32, name="z_sb", tag="z_sb", bufs=2)
                    nc.vector.tensor_copy(z_sb[:], pz[:])
                    rz = work.tile([1, 512], F32, name="rz", tag="rz", bufs=2)
                    nc.vector.reciprocal(rz[:], z_sb[:])
                    rzr = work.tile([1, 512], F32R, name="rzr", tag="rzr", bufs=2)
                    nc.vector.tensor_copy(rzr[:], rz[:])
                    pzb = ps_z.tile([128, 512], F32, name="pzb", tag="pz")
                    nc.tensor.matmul(pzb[:], ones_r32[:], rzr[:],
                                     start=True, stop=True)
                    rzb = work.tile([128, 512], F32, name="rzb", tag="rzb",
                                    bufs=2)
                    nc.vector.tensor_copy(rzb[:], pzb[:])
                    if prev is not None:
                        emit_outproj(*prev)
                    prev = (qb, att, rzb)
                emit_outproj(*prev)

    _split_excess_waits(nc)
    return nc


_cache = {}


def _get_program(with_bv):
    key = ("nc", with_bv)
    if key not in _cache:
        _cache[key] = _build(with_bv)
    return _cache[key]


def kernel(x, gamma, beta, wq, bq, wk, bk, wv, bv, wo, bo, trace=False):
    x = np.asarray(x, dtype=np.float32)
    gamma = np.asarray(gamma, dtype=np.float32)
    beta = np.asarray(beta, dtype=np.float32)
    wq, wk, wv, wo = (np.asarray(a, dtype=np.float32) for a in (wq, wk, wv, wo))
    bq, bk, bv, bo = (np.asarray(a, dtype=np.float32) for a in (bq, bk, bv, bo))

    nc = _get_program(with_bv=bool(np.any(bv)))

    shared = {
        "wqT": np.ascontiguousarray(wq.T).astype(np.float16),
        "wkT": np.ascontiguousarray(wk.T).astype(np.float16),
        "wvT": np.ascontiguousarray(wv.T).astype(np.float16),
        "woT": np.ascontiguousarray(wo.T).astype(np.float16),
        "bqc": np.ascontiguousarray(bq.reshape(CC, 128).T),
        "bkc": np.ascontiguousarray(bk.reshape(CC, 128).T),
        "boc": np.ascontiguousarray(bo.reshape(CC, 128).T),
        "bv16": bv.reshape(1, C).astype(np.float16),
        "gammac": np.ascontiguousarray(gamma.reshape(CC, 128).T),
        "betac": np.ascontiguousarray(beta.reshape(CC, 128).T),
    }
    ind = np.zeros((128, CC, NG), np.float32)
    indT = np.zeros((NG, CC, 128), np.float32)
    for i in range(CC):
        for p in range(128):
            g = (i * 128 + p) // GS
            ind[p, i, g] = 1.0
            indT[g, i, p] = 1.0
    shared["ind"] = ind
    shared["indT"] = indT

    in_maps = []
    for core in range(8):
        b, half = core // 2, core % 2
        xs = x[b].reshape(C, S)
        if half:
            xin = np.concatenate([xs[:, SQ:], xs[:, :SQ]], axis=1)
        else:
            xin = np.ascontiguousarray(xs)
        in_maps.append({"x": xin, **shared})

    res = run_bass_kernel_spmd(nc, in_maps, core_ids=list(range(8)),
                               trace=trace)
    _cache["last_exec_time_ns"] = res.exec_time_ns

    y = np.empty((B, C, S), np.float32)
    for core in range(8):
        b, half = core // 2, core % 2
        y[b, :, half * SQ:(half + 1) * SQ] = \
            res.results[core]["out"].reshape(C, SQ)
    return y.reshape(B, C, H, W)


# revision 20
# speedup vs baseline: 1.0833x; 1.0329x over previous
"""AttnBlock (GroupNorm + single-head spatial attention + residual) on 8
Trainium2 NeuronCores.

Sharding: data-parallel over B (4 batches) x 2-way query-sequence parallel =
8 shards. Each core gets the full x[b] (rolled so its query half is the
first 2048 spatial positions), computes GroupNorm + Q/K/V projections +
attention for its 2048 queries + output projection + residual, and writes a
[512, 2048] slice of the output.

Compute layout (per core, C=512, S=4096, Sq=2048):
  x        [c, s]   4 chunks of [128, 4096] f32 (channels on partitions)
  h = GN(x)         4 chunks of [128, 4096] f16
  q = Wq h + bq     [128, 2048] f16 x4 (out-channels on partitions)
  k = Wk h + bk     [128, 4096] f16 x4
  vT = h^T Wv^T+bv  32 tiles of [128, 512] f16 (spatial on partitions!)
  scoresT[s,q] = k^T q   computed per (128-key-tile x 512-query-block) in
                 PSUM, exp()'d on ScalarE into SBUF f16 -- no transposes
                 anywhere: both AV operands already have s on partitions.
  out'[c,q] += vT^T e    accumulated over all 32 key tiles in 4 PSUM banks
  Z[q]     += 1^T e      (ones-matmul row)
  attn = out'/Z, proj = Wo attn + bo, out = x[:, :2048] + proj

GroupNorm stats use bn_stats/bn_aggr per channel + tiny indicator matmuls to
reduce/broadcast across the 16 channels of each group (cross-partition).

All heavy matmuls run in fp16 (1 PE cycle/row vs 4 for fp32); fp32 would be
~4x slower and fp16 end-to-end error is ~1e-4 of absmax (validated against
the fp32 reference).
"""
import numpy as np

import bass_rust
import concourse.bass as bass
import concourse.tile as tile
from concourse import mybir
from concourse.bass_utils import run_bass_kernel_spmd

F32 = mybir.dt.float32
F32R = mybir.dt.float32r
F16 = mybir.dt.float16
AF = mybir.ActivationFunctionType
ALU = mybir.AluOpType

B, C, H, W = 4, 512, 64, 64
S = H * W            # 4096 spatial positions (keys)
SQ = S // 2          # 2048 queries per core
CC = C // 128        # 4 channel chunks
ST = S // 128        # 32 key tiles
QB = SQ // 512       # 4 query blocks
NG = 32              # groups
GS = C // NG         # 16 channels per group
EPS = 1e-6
SCALE = 1.0 / float(np.sqrt(C))
VSCALE = 2.0 ** -6   # pre-scale on v and the Z-ones so |att| stays in f16


def _split_excess_waits(nc, max_waits=1):
    """walrus in this toolchain rejects instructions with >1 sync-wait.
    Hoist excess waits onto same-engine NOPs placed just before the
    instruction (engine streams are in-order, so this is equivalent)."""
    for f in nc.m.functions:
        for bb in f.blocks:
            out = []
            for inst in bb.instructions:
                si = inst.sync_info
                if si is not None and len(si.on_wait) > max_waits:
                    waits = list(si.on_wait)
                    plain = [w for w in waits if w.wait_reg is None]
                    special = [w for w in waits if w.wait_reg is not None]
                    n_keep = max(0, max_waits - len(special))
                    hoist = plain[: len(plain) - n_keep] if n_keep < len(plain) else []
                    keep = plain[len(hoist):] + special
                    if len(keep) > max_waits:
                        out.append(inst)
                        continue
                    for j, w in enumerate(hoist):
                        nop = mybir.InstNoOp(name=f"{inst.name}-wsplit{j}")
                        nop.engine = inst.engine
                        nop.sync_info = bass_rust.SyncInfo(on_wait=[w], on_update=[])
                        out.append(nop)
                    inst.sync_info = bass_rust.SyncInfo(
                        on_wait=keep, on_update=list(si.on_update))
                out.append(inst)
            bb.instructions = out


def _build(with_bv=True):
    nc = bass.Bass(trn_type="TRN2")

    x_d = nc.dram_tensor("x", [C, S], F32, kind="ExternalInput")
    w_d = {n: nc.dram_tensor(n, [C, C], F16, kind="ExternalInput")
           for n in ("wqT", "wkT", "wvT", "woT")}
    bq_d = nc.dram_tensor("bqc", [128, CC], F32, kind="ExternalInput")
    bk_d = nc.dram_tensor("bkc", [128, CC], F32, kind="ExternalInput")
    bo_d = nc.dram_tensor("boc", [128, CC], F32, kind="ExternalInput")
    bv_d = nc.dram_tensor("bv16", [1, C], F16, kind="ExternalInput")
    ga_d = nc.dram_tensor("gammac", [128, CC], F32, kind="ExternalInput")
    be_d = nc.dram_tensor("betac", [128, CC], F32, kind="ExternalInput")
    ind_d = nc.dram_tensor("ind", [128, CC, NG], F32, kind="ExternalInput")
    indT_d = nc.dram_tensor("indT", [NG, CC, 128], F32, kind="ExternalInput")
    out_d = nc.dram_tensor("out", [CC, 128, SQ], F32, kind="ExternalOutput")

    with tile.TileContext(nc) as tc:
        from contextlib import ExitStack
        with ExitStack() as stack:
            const = stack.enter_context(tc.tile_pool(name="const", bufs=1))
            work = stack.enter_context(tc.tile_pool(name="work", bufs=3))
            p_res = stack.enter_context(tc.tile_pool(name="p_res", bufs=1))
            p_h = stack.enter_context(tc.tile_pool(name="p_h", bufs=1))

            # ---- constants ----
            w_sb = {}
            for n in ("wqT", "wkT", "wvT", "woT"):
                t = const.tile([128, CC, C], F16, name=f"{n}_sb")
                nc.sync.dma_start(out=t[:], in_=w_d[n].rearrange(
                    "(c p) o -> p c o", p=128))
                w_sb[n] = t
            bq_sb = const.tile([128, CC], F32, name="bq_sb")
            nc.sync.dma_start(out=bq_sb[:], in_=bq_d[:, :])
            bk_sb = const.tile([128, CC], F32, name="bk_sb")
            nc.sync.dma_start(out=bk_sb[:], in_=bk_d[:, :])
            bo_sb = const.tile([128, CC], F32, name="bo_sb")
            nc.sync.dma_start(out=bo_sb[:], in_=bo_d[:, :])
            bv_sb = const.tile([1, C], F16, name="bv_sb")
            nc.sync.dma_start(out=bv_sb[:], in_=bv_d[:, :])
            ga_sb = const.tile([128, CC], F32, name="ga_sb")
            nc.sync.dma_start(out=ga_sb[:], in_=ga_d[:, :])
            be_sb = const.tile([128, CC], F32, name="be_sb")
            nc.sync.dma_start(out=be_sb[:], in_=be_d[:, :])
            ind_sb = const.tile([128, CC, NG], F32, name="ind_sb")
            nc.sync.dma_start(out=ind_sb[:], in_=ind_d[:, :, :])
            indT_sb = const.tile([NG, CC, 128], F32, name="indT_sb")
            nc.sync.dma_start(out=indT_sb[:], in_=indT_d[:, :, :])

            ones_r16 = const.tile([1, 128], F16, name="ones_r16")
            nc.vector.memset(ones_r16[:], 1.0)
            ones_c16 = const.tile([128, 1], F16, name="ones_c16")
            nc.vector.memset(ones_c16[:], VSCALE)
            ones_r32 = const.tile([1, 128], F32R, name="ones_r32")
            nc.vector.memset(ones_r32[:].bitcast(F32), 1.0)
            eps_sb = const.tile([NG, 1], F32, name="eps_sb")
            nc.vector.memset(eps_sb[:], EPS)

            h16 = p_h.tile([128, CC, S], F16, name="h16")
            xres = p_res.tile([128, CC, SQ], F32, name="xres")

            # =========== Phase 1: load x + GroupNorm ===========
            with tc.tile_pool(name="p_x", bufs=1) as p_x, \
                 tc.tile_pool(name="ps_gn", bufs=2, space="PSUM") as ps_gn:
                xc = p_x.tile([128, CC, S], F32, name="xc")
                for i in range(CC):
                    for hh in range(2):
                        cols = slice(hh * SQ, (hh + 1) * SQ)
                        nc.sync.dma_start(
                            out=xc[:, i, cols],
                            in_=x_d[i * 128:(i + 1) * 128, cols])

                # per-channel [sum, sumsq]: sum on DVE (reduce), sumsq on
                # ScalarE (Square + accum_out) -- the two run in parallel
                stats2 = []
                for i in range(CC):
                    s2 = work.tile([128, 2], F32, name="s2", tag="gn_s2", bufs=4)
                    nc.vector.tensor_reduce(out=s2[:, 0:1], in_=xc[:, i, :],
                                            axis=mybir.AxisListType.X,
                                            op=ALU.add)
                    sq = p_x.tile([128, S], F32, name="sq", tag="sq", bufs=1)
                    nc.scalar.activation(sq[:], xc[:, i, :], AF.Square,
                                         accum_out=s2[:, 1:2])
                    stats2.append(s2)

                # copy residual half out before x dies (ScalarE is idle
                # after the squares; keeps DVE free for the stats chain)
                for i in range(CC):
                    nc.scalar.copy(xres[:, i, :], xc[:, i, :SQ])

                # reduce over the 16 channels of each group: indicator matmul
                psg = ps_gn.tile([NG, 2], F32, name="psg")
                for i in range(CC):
                    nc.tensor.matmul(psg[:], ind_sb[:, i, :], stats2[i][:],
                                     start=(i == 0), stop=(i == CC - 1))
                gstat = work.tile([NG, 2], F32, name="gstat")
                nc.scalar.mul(gstat[:], psg[:], 1.0 / (GS * S))  # [mean_g, E_g[x^2]]

                # var_g, rstd_g (+ one Newton step to clean up ACT sqrt)
                mg2 = work.tile([NG, 1], F32, name="mg2")
                nc.vector.tensor_mul(mg2[:], gstat[:, 0:1], gstat[:, 0:1])
                varg = work.tile([NG, 1], F32, name="varg")
                nc.vector.tensor_tensor(out=varg[:], in0=gstat[:, 1:2],
                                        in1=mg2[:], op=ALU.subtract)
                sd = work.tile([NG, 1], F32, name="sd")
                nc.scalar.activation(sd[:], varg[:], AF.Sqrt, bias=eps_sb[:])
                r0 = work.tile([NG, 1], F32, name="r0")
                nc.vector.reciprocal(r0[:], sd[:])
                ve = work.tile([NG, 1], F32, name="ve")
                nc.vector.tensor_scalar_add(ve[:], varg[:], EPS)
                r0sq = work.tile([NG, 1], F32, name="r0sq")
                nc.vector.tensor_mul(r0sq[:], r0[:], r0[:])
                t2 = work.tile([NG, 1], F32, name="t2")
                nc.vector.tensor_mul(t2[:], ve[:], r0sq[:])
                t3 = work.tile([NG, 1], F32, name="t3")
                nc.vector.tensor_scalar(out=t3[:], in0=t2[:], scalar1=-0.5,
                                        scalar2=1.5, op0=ALU.mult, op1=ALU.add)
                gv = work.tile([NG, 2], F32, name="gv")  # [mean_g, rstd_g]
                nc.vector.tensor_copy(gv[:, 0:1], gstat[:, 0:1])
                nc.vector.tensor_mul(gv[:, 1:2], r0[:], t3[:])

                # broadcast group stats back to channels, fold gamma/beta
                for i in range(CC):
                    psb = ps_gn.tile([128, 2], F32, name="psb")
                    nc.tensor.matmul(psb[:], indT_sb[:, i, :], gv[:],
                                     start=True, stop=True)
                    mr = work.tile([128, 2], F32, name="mr", tag="gn_mr", bufs=4)
                    nc.scalar.copy(mr[:], psb[:])
                    sc_c = work.tile([128, 1], F32, name="sc_c", tag="gn_sc", bufs=4)
                    nc.vector.tensor_mul(sc_c[:], mr[:, 1:2], ga_sb[:, i:i + 1])
                    mt = work.tile([128, 1], F32, name="mt", tag="gn_mt", bufs=4)
                    nc.vector.tensor_mul(mt[:], mr[:, 0:1], sc_c[:])
                    bi_c = work.tile([128, 1], F32, name="bi_c", tag="gn_bi", bufs=4)
                    nc.vector.tensor_tensor(out=bi_c[:], in0=be_sb[:, i:i + 1],
                                            in1=mt[:], op=ALU.subtract)
                    # h = x*scale + bias, cast to f16
                    nc.vector.tensor_scalar(
                        out=h16[:, i, :], in0=xc[:, i, :],
                        scalar1=sc_c[:], scalar2=bi_c[:],
                        op0=ALU.mult, op1=ALU.add)

            # =========== Phase 2: projections ===========
            p_kv = stack.enter_context(tc.tile_pool(name="p_kv", bufs=1))
            k16 = p_kv.tile([128, CC, S], F16, name="k16")
            q16 = p_kv.tile([128, CC, SQ], F16, name="q16")
            vT16 = p_kv.tile([128, ST, C], F16, name="vT16")

            with tc.tile_pool(name="ps_proj", bufs=3, space="PSUM") as ps_p:
                # q = WqT^T h (+bq): only the first SQ columns of h
                for oc in range(CC):
                    for qb in range(SQ // 512):
                        pt = ps_p.tile([128, 512], F32, name="pt", tag="pp")
                        cols = slice(qb * 512, (qb + 1) * 512)
                        for ic in range(CC):
                            nc.tensor.matmul(
                                pt[:], w_sb["wqT"][:, ic, oc * 128:(oc + 1) * 128],
                                h16[:, ic, cols],
                                start=(ic == 0), stop=(ic == CC - 1))
                        nc.scalar.activation(q16[:, oc, cols], pt[:],
                                             AF.Identity, bias=bq_sb[:, oc:oc + 1])
                # k = WkT^T h (+bk): all S columns
                for oc in range(CC):
                    for sb in range(S // 512):
                        pt = ps_p.tile([128, 512], F32, name="pt", tag="pp")
                        cols = slice(sb * 512, (sb + 1) * 512)
                        for ic in range(CC):
                            nc.tensor.matmul(
                                pt[:], w_sb["wkT"][:, ic, oc * 128:(oc + 1) * 128],
                                h16[:, ic, cols],
                                start=(ic == 0), stop=(ic == CC - 1))
                        nc.scalar.activation(k16[:, oc, cols], pt[:],
                                             AF.Identity, bias=bk_sb[:, oc:oc + 1])
                # vT[s, c] = h[:, s]^T WvT (+bv broadcast via ones-matmul).
                # vT is stored pre-scaled by 2^-6 (and the Z-ones column uses
                # the same scale) so the unnormalized attention accumulator
                # stays comfortably inside f16 range; the scale cancels in
                # the final (Wo att)/Z normalization.
                for st in range(ST):
                    pt = ps_p.tile([128, 512], F32, name="pt", tag="pp")
                    scols = slice(st * 128, (st + 1) * 128)
                    for ic in range(CC):
                        nc.tensor.matmul(pt[:], h16[:, ic, scols],
                                         w_sb["wvT"][:, ic, :],
                                         start=(ic == 0),
                                         stop=(ic == CC - 1 and not with_bv))
                    if with_bv:
                        nc.tensor.matmul(pt[:], ones_r16[:], bv_sb[:],
                                         start=False, stop=True)
                    nc.scalar.mul(vT16[:, st, :], pt[:], VSCALE)

            # =========== Phase 3: attention + out-projection ===========
            # att (= 2^-6 * sum_s e[s,q] v[:,s], unnormalized) is evacuated
            # to f16 right after the key loop; normalization by 1/Z happens
            # AFTER the out-projection (it commutes with Wo), so the
            # reciprocal/broadcast chain runs on DVE off the PE critical
            # path. The out-projection for block qb is emitted after block
            # qb+1's key loop so its PSUM->f16 dependency is fully hidden.
            with tc.tile_pool(name="ps_po", bufs=4, space="PSUM") as ps_po, \
                 tc.tile_pool(name="ps_z", bufs=2, space="PSUM") as ps_z, \
                 tc.tile_pool(name="ps_s", bufs=2, space="PSUM") as ps_s:

                def emit_outproj(qb, att, rzr):
                    qcols = slice(qb * 512, (qb + 1) * 512)
                    # broadcast 1/Z across partitions (rzr computed long ago,
                    # so this matmul never stalls the PE stream)
                    pzb = ps_z.tile([128, 512], F32, name="pzb", tag="pz")
                    nc.tensor.matmul(pzb[:], ones_r32[:], rzr[:],
                                     start=True, stop=True)
                    rzb = work.tile([128, 512], F32, name="rzb", tag="rzb",
                                    bufs=2)
                    nc.vector.tensor_copy(rzb[:], pzb[:])
                    for oc in range(CC):
                        pp = ps_s.tile([128, 512], F32, name="pp", tag="msum")
                        for cc2 in range(CC):
                            nc.tensor.matmul(
                                pp[:],
                                w_sb["woT"][:, cc2, oc * 128:(oc + 1) * 128],
                                att[cc2][:],
                                start=(cc2 == 0), stop=(cc2 == CC - 1))
                        t32 = work.tile([128, 512], F32, name="t32", tag="t32", bufs=2)
                        nc.vector.tensor_mul(t32[:], pp[:], rzb[:])
                        o32 = work.tile([128, 512], F32, name="o32", tag="o32", bufs=2)
                        nc.vector.scalar_tensor_tensor(
                            out=o32[:], in0=t32[:], scalar=bo_sb[:, oc:oc + 1],
                            in1=xres[:, oc, qcols], op0=ALU.add, op1=ALU.add)
                        nc.sync.dma_start(out=out_d[oc, :, qcols], in_=o32[:])

                def emit_scores(qb, st):
                    qcols = slice(qb * 512, (qb + 1) * 512)
                    pscore = ps_s.tile([128, 512], F32, name="pscore",
                                       tag="msum")
                    scols = slice(st * 128, (st + 1) * 128)
                    for ic in range(CC):
                        nc.tensor.matmul(pscore[:], k16[:, ic, scols],
                                         q16[:, ic, qcols],
                                         start=(ic == 0), stop=(ic == CC - 1))
                    e16 = work.tile([128, 512], F16, name="e16",
                                    tag="e16", bufs=4)
                    nc.scalar.activation(e16[:], pscore[:], AF.Exp,
                                         scale=SCALE)
                    return e16

                def emit_av(po, pz, st, e16):
                    for cc2 in range(CC):
                        nc.tensor.matmul(
                            po[cc2][:],
                            vT16[:, st, cc2 * 128:(cc2 + 1) * 128],
                            e16[:],
                            start=(st == 0), stop=(st == ST - 1))
                    nc.tensor.matmul(pz[:], ones_c16[:], e16[:],
                                     start=(st == 0), stop=(st == ST - 1))

                prev = None
                for qb in range(QB):
                    po = [ps_po.tile([128, 512], F32, name="po", tag="po")
                          for _ in range(CC)]
                    pz = ps_z.tile([1, 512], F32, name="pz", tag="pz")
                    # software-pipelined: scores/exp for key-tile st+1 are
                    # issued before the AV matmuls of key-tile st, so the PE
                    # never waits on the ScalarE exp.
                    e_prev = emit_scores(qb, 0)
                    for st in range(1, ST):
                        e_cur = emit_scores(qb, st)
                        emit_av(po, pz, st - 1, e_prev)
                        e_prev = e_cur
                    emit_av(po, pz, ST - 1, e_prev)
                    # evacuate att to f16 (frees po for the next block);
                    # 1/Z on DVE off the PE critical path
                    att = [work.tile([128, 512], F16, name="att",
                                     tag="att", bufs=8) for _ in range(CC)]
                    for cc2 in range(CC):
                        nc.vector.tensor_copy(att[cc2][:], po[cc2][:])
                    z_sb = work.tile([1, 512], F32, name="z_sb", tag="z_sb", bufs=2)
                    nc.vector.tensor_copy(z_sb[:], pz[:])
                    rz = work.tile([1, 512], F32, name="rz", tag="rz", bufs=2)
                    nc.vector.reciprocal(rz[:], z_sb[:])
                    rzr = work.tile([1, 512], F32R, name="rzr", tag="rzr", bufs=2)
                    nc.vector.tensor_copy(rzr[:], rz[:])
                    if prev is not None:
                        emit_outproj(*prev)
                    prev = (qb, att, rzr)
                emit_outproj(*prev)

    _split_excess_waits(nc)
    return nc


_cache = {}


def _get_program(with_bv):
    key = ("nc", with_bv)
    if key not in _cache:
        _cache[key] = _build(with_bv)
    return _cache[key]


def kernel(x, gamma, beta, wq, bq, wk, bk, wv, bv, wo, bo, trace=False):
    x = np.asarray(x, dtype=np.float32)
    gamma = np.asarray(gamma, dtype=np.float32)
    beta = np.asarray(beta, dtype=np.float32)
    wq, wk, wv, wo = (np.asarray(a, dtype=np.float32) for a in (wq, wk, wv, wo))
    bq, bk, bv, bo = (np.asarray(a, dtype=np.float32) for a in (bq, bk, bv, bo))

    nc = _get_program(with_bv=bool(np.any(bv)))

    shared = {
        "wqT": np.ascontiguousarray(wq.T).astype(np.float16),
        "wkT": np.ascontiguousarray(wk.T).astype(np.float16),
        "wvT": np.ascontiguousarray(wv.T).astype(np.float16),
        "woT": np.ascontiguousarray(wo.T).astype(np.float16),
        "bqc": np.ascontiguousarray(bq.reshape(CC, 128).T),
        "bkc": np.ascontiguousarray(bk.reshape(CC, 128).T),
        "boc": np.ascontiguousarray(bo.reshape(CC, 128).T),
        "bv16": bv.reshape(1, C).astype(np.float16),
        "gammac": np.ascontiguousarray(gamma.reshape(CC, 128).T),
        "betac": np.ascontiguousarray(beta.reshape(CC, 128).T),
    }
    ind = np.zeros((128, CC, NG), np.float32)
    indT = np.zeros((NG, CC, 128), np.float32)
    for i in range(CC):
        for p in range(128):
            g = (i * 128 + p) // GS
            ind[p, i, g] = 1.0
            indT[g, i, p] = 1.0
    shared["ind"] = ind
    shared["indT"] = indT

    in_maps = []
    for core in range(8):
        b, half = core // 2, core % 2
        xs = x[b].reshape(C, S)
        if half:
            xin = np.concatenate([xs[:, SQ:], xs[:, :SQ]], axis=1)
        else:
            xin = np.ascontiguousarray(xs)
        in_maps.append({"x": xin, **shared})

    res = run_bass_kernel_spmd(nc, in_maps, core_ids=list(range(8)),
                               trace=trace)
    _cache["last_exec_time_ns"] = res.exec_time_ns

    y = np.empty((B, C, S), np.float32)
    for core in range(8):
        b, half = core // 2, core % 2
        y[b, :, half * SQ:(half + 1) * SQ] = \
            res.results[core]["out"].reshape(C, SQ)
    return y.reshape(B, C, H, W)


# revision 25
# speedup vs baseline: 1.1505x; 1.0621x over previous
"""AttnBlock (GroupNorm + single-head spatial attention + residual) on 8
Trainium2 NeuronCores.

Sharding: data-parallel over B (4 batches) x 2-way query-sequence parallel =
8 shards. Each core gets the full x[b] (rolled so its query half is the
first 2048 spatial positions), computes GroupNorm + Q/K/V projections +
attention for its 2048 queries + output projection + residual, and writes a
[512, 2048] slice of the output.

Compute layout (per core, C=512, S=4096, Sq=2048):
  x        [c, s]   4 chunks of [128, 4096] f32 (channels on partitions)
  h = GN(x)         4 chunks of [128, 4096] f16
  q = Wq h + bq     [128, 2048] f16 x4 (out-channels on partitions)
  k = Wk h + bk     [128, 4096] f16 x4
  vT = h^T Wv^T+bv  32 tiles of [128, 512] f16 (spatial on partitions!)
  scoresT[s,q] = k^T q   computed per (128-key-tile x 512-query-block) in
                 PSUM, exp()'d on ScalarE into SBUF f16 -- no transposes
                 anywhere: both AV operands already have s on partitions.
  out'[c,q] += vT^T e    accumulated over all 32 key tiles in 4 PSUM banks
  Z[q]     += 1^T e      (ones-matmul row)
  attn = out'/Z, proj = Wo attn + bo, out = x[:, :2048] + proj

GroupNorm stats use bn_stats/bn_aggr per channel + tiny indicator matmuls to
reduce/broadcast across the 16 channels of each group (cross-partition).

All heavy matmuls run in fp16 (1 PE cycle/row vs 4 for fp32); fp32 would be
~4x slower and fp16 end-to-end error is ~1e-4 of absmax (validated against
the fp32 reference).
"""
import numpy as np

import bass_rust
import concourse.bass as bass
import concourse.tile as tile
from concourse import mybir
from concourse.bass_utils import run_bass_kernel_spmd

F32 = mybir.dt.float32
F32R = mybir.dt.float32r
F16 = mybir.dt.float16
AF = mybir.ActivationFunctionType
ALU = mybir.AluOpType

B, C, H, W = 4, 512, 64, 64
S = H * W            # 4096 spatial positions (keys)
SQ = S // 2          # 2048 queries per core
CC = C // 128        # 4 channel chunks
ST = S // 128        # 32 key tiles
QB = SQ // 512       # 4 query blocks
NG = 32              # groups
GS = C // NG         # 16 channels per group
EPS = 1e-6
SCALE = 1.0 / float(np.sqrt(C))
VSCALE = 2.0 ** -6   # pre-scale on v and the Z-ones so |att| stays in f16


def _split_excess_waits(nc, max_waits=1):
    """walrus in this toolchain rejects instructions with >1 sync-wait.
    Hoist excess waits onto same-engine NOPs placed just before the
    instruction (engine streams are in-order, so this is equivalent)."""
    for f in nc.m.functions:
        for bb in f.blocks:
            out = []
            for inst in bb.instructions:
                si = inst.sync_info
                if si is not None and len(si.on_wait) > max_waits:
                    waits = list(si.on_wait)
                    plain = [w for w in waits if w.wait_reg is None]
                    special = [w for w in waits if w.wait_reg is not None]
                    n_keep = max(0, max_waits - len(special))
                    hoist = plain[: len(plain) - n_keep] if n_keep < len(plain) else []
                    keep = plain[len(hoist):] + special
                    if len(keep) > max_waits:
                        out.append(inst)
                        continue
                    for j, w in enumerate(hoist):
                        nop = mybir.InstNoOp(name=f"{inst.name}-wsplit{j}")
                        nop.engine = inst.engine
                        nop.sync_info = bass_rust.SyncInfo(on_wait=[w], on_update=[])
                        out.append(nop)
                    inst.sync_info = bass_rust.SyncInfo(
                        on_wait=keep, on_update=list(si.on_update))
                out.append(inst)
            bb.instructions = out


def _build(with_bv=True):
    nc = bass.Bass(trn_type="TRN2")

    x_d = nc.dram_tensor("x", [C, S], F32, kind="ExternalInput")
    w_d = {n: nc.dram_tensor(n, [C, C], F16, kind="ExternalInput")
           for n in ("wqT", "wkT", "wvT", "woT")}
    bq_d = nc.dram_tensor("bqc", [128, CC], F32, kind="ExternalInput")
    bk_d = nc.dram_tensor("bkc", [128, CC], F32, kind="ExternalInput")
    bo_d = nc.dram_tensor("boc", [128, CC], F32, kind="ExternalInput")
    bv_d = nc.dram_tensor("bv16", [1, C], F16, kind="ExternalInput")
    ga_d = nc.dram_tensor("gammac", [128, CC], F32, kind="ExternalInput")
    be_d = nc.dram_tensor("betac", [128, CC], F32, kind="ExternalInput")
    ind_d = nc.dram_tensor("ind", [128, CC, NG], F32, kind="ExternalInput")
    indT_d = nc.dram_tensor("indT", [NG, CC, 128], F32, kind="ExternalInput")
    out_d = nc.dram_tensor("out", [CC, 128, SQ], F32, kind="ExternalOutput")

    with tile.TileContext(nc) as tc:
        from contextlib import ExitStack
        with ExitStack() as stack:
            const = stack.enter_context(tc.tile_pool(name="const", bufs=1))
            work = stack.enter_context(tc.tile_pool(name="work", bufs=3))
            p_res = stack.enter_context(tc.tile_pool(name="p_res", bufs=1))
            p_h = stack.enter_context(tc.tile_pool(name="p_h", bufs=1))

            # ---- constants ----
            w_sb = {}
            for n in ("wqT", "wkT", "wvT", "woT"):
                t = const.tile([128, CC, C], F16, name=f"{n}_sb")
                nc.sync.dma_start(out=t[:], in_=w_d[n].rearrange(
                    "(c p) o -> p c o", p=128))
                w_sb[n] = t
            bq_sb = const.tile([128, CC], F32, name="bq_sb")
            nc.sync.dma_start(out=bq_sb[:], in_=bq_d[:, :])
            bk_sb = const.tile([128, CC], F32, name="bk_sb")
            nc.sync.dma_start(out=bk_sb[:], in_=bk_d[:, :])
            bo_sb = const.tile([128, CC], F32, name="bo_sb")
            nc.sync.dma_start(out=bo_sb[:], in_=bo_d[:, :])
            bv_sb = const.tile([1, C], F16, name="bv_sb")
            nc.sync.dma_start(out=bv_sb[:], in_=bv_d[:, :])
            ga_sb = const.tile([128, CC], F32, name="ga_sb")
            nc.sync.dma_start(out=ga_sb[:], in_=ga_d[:, :])
            be_sb = const.tile([128, CC], F32, name="be_sb")
            nc.sync.dma_start(out=be_sb[:], in_=be_d[:, :])
            ind_sb = const.tile([128, CC, NG], F32, name="ind_sb")
            nc.sync.dma_start(out=ind_sb[:], in_=ind_d[:, :, :])
            indT_sb = const.tile([NG, CC, 128], F32, name="indT_sb")
            nc.sync.dma_start(out=indT_sb[:], in_=indT_d[:, :, :])

            ones_r16 = const.tile([1, 128], F16, name="ones_r16")
            nc.vector.memset(ones_r16[:], 1.0)
            # full-width ones: the Z matmul keeps the PE's fast-weight-load
            # mode (a 1-col lhsT would break FWL and tax neighboring MMs),
            # and its PSUM output is Z broadcast across all 128 partitions.
            ones_sq16 = const.tile([128, 128], F16, name="ones_sq16")
            nc.vector.memset(ones_sq16[:], VSCALE)
            eps_sb = const.tile([NG, 1], F32, name="eps_sb")
            nc.vector.memset(eps_sb[:], EPS)

            h16 = p_h.tile([128, CC, S], F16, name="h16")
            xres = p_res.tile([128, CC, SQ], F32, name="xres")

            # =========== Phase 1: load x + GroupNorm ===========
            with tc.tile_pool(name="p_x", bufs=1) as p_x, \
                 tc.tile_pool(name="ps_gn", bufs=2, space="PSUM") as ps_gn:
                xc = p_x.tile([128, CC, S], F32, name="xc")
                for i in range(CC):
                    for hh in range(2):
                        cols = slice(hh * SQ, (hh + 1) * SQ)
                        nc.sync.dma_start(
                            out=xc[:, i, cols],
                            in_=x_d[i * 128:(i + 1) * 128, cols])

                # per-channel [sum, sumsq]: sum on DVE (reduce), sumsq on
                # ScalarE (Square + accum_out) -- the two run in parallel
                stats2 = []
                for i in range(CC):
                    s2 = work.tile([128, 2], F32, name="s2", tag="gn_s2", bufs=4)
                    nc.vector.tensor_reduce(out=s2[:, 0:1], in_=xc[:, i, :],
                                            axis=mybir.AxisListType.X,
                                            op=ALU.add)
                    sq = p_x.tile([128, S], F32, name="sq", tag="sq", bufs=1)
                    nc.scalar.activation(sq[:], xc[:, i, :], AF.Square,
                                         accum_out=s2[:, 1:2])
                    stats2.append(s2)

                # copy residual half out before x dies (ScalarE is idle
                # after the squares; keeps DVE free for the stats chain)
                for i in range(CC):
                    nc.scalar.copy(xres[:, i, :], xc[:, i, :SQ])

                # reduce over the 16 channels of each group: indicator matmul
                psg = ps_gn.tile([NG, 2], F32, name="psg")
                for i in range(CC):
                    nc.tensor.matmul(psg[:], ind_sb[:, i, :], stats2[i][:],
                                     start=(i == 0), stop=(i == CC - 1))
                gstat = work.tile([NG, 2], F32, name="gstat")
                nc.scalar.mul(gstat[:], psg[:], 1.0 / (GS * S))  # [mean_g, E_g[x^2]]

                # var_g, rstd_g (+ one Newton step to clean up ACT sqrt)
                mg2 = work.tile([NG, 1], F32, name="mg2")
                nc.vector.tensor_mul(mg2[:], gstat[:, 0:1], gstat[:, 0:1])
                varg = work.tile([NG, 1], F32, name="varg")
                nc.vector.tensor_tensor(out=varg[:], in0=gstat[:, 1:2],
                                        in1=mg2[:], op=ALU.subtract)
                sd = work.tile([NG, 1], F32, name="sd")
                nc.scalar.activation(sd[:], varg[:], AF.Sqrt, bias=eps_sb[:])
                r0 = work.tile([NG, 1], F32, name="r0")
                nc.vector.reciprocal(r0[:], sd[:])
                ve = work.tile([NG, 1], F32, name="ve")
                nc.vector.tensor_scalar_add(ve[:], varg[:], EPS)
                r0sq = work.tile([NG, 1], F32, name="r0sq")
                nc.vector.tensor_mul(r0sq[:], r0[:], r0[:])
                t2 = work.tile([NG, 1], F32, name="t2")
                nc.vector.tensor_mul(t2[:], ve[:], r0sq[:])
                t3 = work.tile([NG, 1], F32, name="t3")
                nc.vector.tensor_scalar(out=t3[:], in0=t2[:], scalar1=-0.5,
                                        scalar2=1.5, op0=ALU.mult, op1=ALU.add)
                gv = work.tile([NG, 2], F32, name="gv")  # [mean_g, rstd_g]
                nc.vector.tensor_copy(gv[:, 0:1], gstat[:, 0:1])
                nc.vector.tensor_mul(gv[:, 1:2], r0[:], t3[:])

                # broadcast group stats back to channels, fold gamma/beta
                for i in range(CC):
                    psb = ps_gn.tile([128, 2], F32, name="psb")
                    nc.tensor.matmul(psb[:], indT_sb[:, i, :], gv[:],
                                     start=True, stop=True)
                    mr = work.tile([128, 2], F32, name="mr", tag="gn_mr", bufs=4)
                    nc.scalar.copy(mr[:], psb[:])
                    sc_c = work.tile([128, 1], F32, name="sc_c", tag="gn_sc", bufs=4)
                    nc.vector.tensor_mul(sc_c[:], mr[:, 1:2], ga_sb[:, i:i + 1])
                    mt = work.tile([128, 1], F32, name="mt", tag="gn_mt", bufs=4)
                    nc.vector.tensor_mul(mt[:], mr[:, 0:1], sc_c[:])
                    bi_c = work.tile([128, 1], F32, name="bi_c", tag="gn_bi", bufs=4)
                    nc.vector.tensor_tensor(out=bi_c[:], in0=be_sb[:, i:i + 1],
                                            in1=mt[:], op=ALU.subtract)
                    # h = x*scale + bias, cast to f16
                    nc.vector.tensor_scalar(
                        out=h16[:, i, :], in0=xc[:, i, :],
                        scalar1=sc_c[:], scalar2=bi_c[:],
                        op0=ALU.mult, op1=ALU.add)

            # =========== Phase 2: projections ===========
            p_kv = stack.enter_context(tc.tile_pool(name="p_kv", bufs=1))
            k16 = p_kv.tile([128, CC, S], F16, name="k16")
            q16 = p_kv.tile([128, CC, SQ], F16, name="q16")
            vT16 = p_kv.tile([128, ST, C], F16, name="vT16")

            with tc.tile_pool(name="ps_proj", bufs=3, space="PSUM") as ps_p:
                # q = WqT^T h (+bq): only the first SQ columns of h
                for oc in range(CC):
                    for qb in range(SQ // 512):
                        pt = ps_p.tile([128, 512], F32, name="pt", tag="pp")
                        cols = slice(qb * 512, (qb + 1) * 512)
                        for ic in range(CC):
                            nc.tensor.matmul(
                                pt[:], w_sb["wqT"][:, ic, oc * 128:(oc + 1) * 128],
                                h16[:, ic, cols],
                                start=(ic == 0), stop=(ic == CC - 1))
                        nc.scalar.activation(q16[:, oc, cols], pt[:],
                                             AF.Identity, bias=bq_sb[:, oc:oc + 1])
                # k = WkT^T h (+bk): all S columns
                for oc in range(CC):
                    for sb in range(S // 512):
                        pt = ps_p.tile([128, 512], F32, name="pt", tag="pp")
                        cols = slice(sb * 512, (sb + 1) * 512)
                        for ic in range(CC):
                            nc.tensor.matmul(
                                pt[:], w_sb["wkT"][:, ic, oc * 128:(oc + 1) * 128],
                                h16[:, ic, cols],
                                start=(ic == 0), stop=(ic == CC - 1))
                        nc.scalar.activation(k16[:, oc, cols], pt[:],
                                             AF.Identity, bias=bk_sb[:, oc:oc + 1])
                # vT[s, c] = h[:, s]^T WvT (+bv broadcast via ones-matmul).
                # vT is stored pre-scaled by 2^-6 (and the Z-ones column uses
                # the same scale) so the unnormalized attention accumulator
                # stays comfortably inside f16 range; the scale cancels in
                # the final (Wo att)/Z normalization.
                for st in range(ST):
                    pt = ps_p.tile([128, 512], F32, name="pt", tag="pp")
                    scols = slice(st * 128, (st + 1) * 128)
                    for ic in range(CC):
                        nc.tensor.matmul(pt[:], h16[:, ic, scols],
                                         w_sb["wvT"][:, ic, :],
                                         start=(ic == 0),
                                         stop=(ic == CC - 1 and not with_bv))
                    if with_bv:
                        nc.tensor.matmul(pt[:], ones_r16[:], bv_sb[:],
                                         start=False, stop=True)
                    nc.scalar.mul(vT16[:, st, :], pt[:], VSCALE)

            # =========== Phase 3: attention + out-projection ===========
            # att (= 2^-6 * sum_s e[s,q] v[:,s], unnormalized) is evacuated
            # to f16 right after the key loop; normalization by 1/Z happens
            # AFTER the out-projection (it commutes with Wo), so the
            # reciprocal/broadcast chain runs on DVE off the PE critical
            # path. The out-projection for block qb is emitted after block
            # qb+1's key loop so its PSUM->f16 dependency is fully hidden.
            with tc.tile_pool(name="ps_po", bufs=4, space="PSUM") as ps_po, \
                 tc.tile_pool(name="ps_z", bufs=2, space="PSUM") as ps_z, \
                 tc.tile_pool(name="ps_s", bufs=2, space="PSUM") as ps_s:

                def emit_outproj(qb, att, rzb):
                    qcols = slice(qb * 512, (qb + 1) * 512)
                    for oc in range(CC):
                        pp = ps_s.tile([128, 512], F32, name="pp", tag="msum")
                        for cc2 in range(CC):
                            nc.tensor.matmul(
                                pp[:],
                                w_sb["woT"][:, cc2, oc * 128:(oc + 1) * 128],
                                att[cc2][:],
                                start=(cc2 == 0), stop=(cc2 == CC - 1))
                        t32 = work.tile([128, 512], F32, name="t32", tag="t32", bufs=2)
                        nc.vector.tensor_mul(t32[:], pp[:], rzb[:])
                        o32 = work.tile([128, 512], F32, name="o32", tag="o32", bufs=2)
                        nc.vector.scalar_tensor_tensor(
                            out=o32[:], in0=t32[:], scalar=bo_sb[:, oc:oc + 1],
                            in1=xres[:, oc, qcols], op0=ALU.add, op1=ALU.add)
                        nc.sync.dma_start(out=out_d[oc, :, qcols], in_=o32[:])

                def emit_scores(qb, st):
                    qcols = slice(qb * 512, (qb + 1) * 512)
                    pscore = ps_s.tile([128, 512], F32, name="pscore",
                                       tag="msum")
                    scols = slice(st * 128, (st + 1) * 128)
                    for ic in range(CC):
                        nc.tensor.matmul(pscore[:], k16[:, ic, scols],
                                         q16[:, ic, qcols],
                                         start=(ic == 0), stop=(ic == CC - 1))
                    e16 = work.tile([128, 512], F16, name="e16",
                                    tag="e16", bufs=4)
                    nc.scalar.activation(e16[:], pscore[:], AF.Exp,
                                         scale=SCALE)
                    return e16

                def emit_av(po, pz, st, e16):
                    for cc2 in range(CC):
                        nc.tensor.matmul(
                            po[cc2][:],
                            vT16[:, st, cc2 * 128:(cc2 + 1) * 128],
                            e16[:],
                            start=(st == 0), stop=(st == ST - 1))
                    nc.tensor.matmul(pz[:], ones_sq16[:], e16[:],
                                     start=(st == 0), stop=(st == ST - 1))

                prev = None
                for qb in range(QB):
                    po = [ps_po.tile([128, 512], F32, name="po", tag="po")
                          for _ in range(CC)]
                    pz = ps_z.tile([128, 512], F32, name="pz", tag="pz")
                    # software-pipelined: scores/exp for key-tile st+1 are
                    # issued before the AV matmuls of key-tile st, so the PE
                    # never waits on the ScalarE exp.
                    e_prev = emit_scores(qb, 0)
                    for st in range(1, ST):
                        e_cur = emit_scores(qb, st)
                        emit_av(po, pz, st - 1, e_prev)
                        e_prev = e_cur
                    emit_av(po, pz, ST - 1, e_prev)
                    # evacuate att to f16 (frees po for the next block);
                    # 1/Z on DVE off the PE critical path
                    att = [work.tile([128, 512], F16, name="att",
                                     tag="att", bufs=8) for _ in range(CC)]
                    for cc2 in range(CC):
                        nc.vector.tensor_copy(att[cc2][:], po[cc2][:])
                    zb = work.tile([128, 512], F32, name="zb", tag="zb", bufs=2)
                    nc.vector.tensor_copy(zb[:], pz[:])
                    rzb = work.tile([128, 512], F32, name="rzb", tag="rzb",
                                    bufs=2)
                    nc.vector.reciprocal(rzb[:], zb[:])
                    if prev is not None:
                        emit_outproj(*prev)
                    prev = (qb, att, rzb)
                emit_outproj(*prev)

    _split_excess_waits(nc)
    return nc


_cache = {}


def _get_program(with_bv):
    key = ("nc", with_bv)
    if key not in _cache:
        _cache[key] = _build(with_bv)
    return _cache[key]


def kernel(x, gamma, beta, wq, bq, wk, bk, wv, bv, wo, bo, trace=False):
    x = np.asarray(x, dtype=np.float32)
    gamma = np.asarray(gamma, dtype=np.float32)
    beta = np.asarray(beta, dtype=np.float32)
    wq, wk, wv, wo = (np.asarray(a, dtype=np.float32) for a in (wq, wk, wv, wo))
    bq, bk, bv, bo = (np.asarray(a, dtype=np.float32) for a in (bq, bk, bv, bo))

    nc = _get_program(with_bv=bool(np.any(bv)))

    shared = {
        "wqT": np.ascontiguousarray(wq.T).astype(np.float16),
        "wkT": np.ascontiguousarray(wk.T).astype(np.float16),
        "wvT": np.ascontiguousarray(wv.T).astype(np.float16),
        "woT": np.ascontiguousarray(wo.T).astype(np.float16),
        "bqc": np.ascontiguousarray(bq.reshape(CC, 128).T),
        "bkc": np.ascontiguousarray(bk.reshape(CC, 128).T),
        "boc": np.ascontiguousarray(bo.reshape(CC, 128).T),
        "bv16": bv.reshape(1, C).astype(np.float16),
        "gammac": np.ascontiguousarray(gamma.reshape(CC, 128).T),
        "betac": np.ascontiguousarray(beta.reshape(CC, 128).T),
    }
    ind = np.zeros((128, CC, NG), np.float32)
    indT = np.zeros((NG, CC, 128), np.float32)
    for i in range(CC):
        for p in range(128):
            g = (i * 128 + p) // GS
            ind[p, i, g] = 1.0
            indT[g, i, p] = 1.0
    shared["ind"] = ind
    shared["indT"] = indT

    in_maps = []
    for core in range(8):
        b, half = core // 2, core % 2
        xs = x[b].reshape(C, S)
        if half:
            xin = np.concatenate([xs[:, SQ:], xs[:, :SQ]], axis=1)
        else:
            xin = np.ascontiguousarray(xs)
        in_maps.append({"x": xin, **shared})

    res = run_bass_kernel_spmd(nc, in_maps, core_ids=list(range(8)),
                               trace=trace)
    _cache["last_exec_time_ns"] = res.exec_time_ns

    y = np.empty((B, C, S), np.float32)
    for core in range(8):
        b, half = core // 2, core % 2
        y[b, :, half * SQ:(half + 1) * SQ] = \
            res.results[core]["out"].reshape(C, SQ)
    return y.reshape(B, C, H, W)


# revision 30
# speedup vs baseline: 1.1862x; 1.0310x over previous
"""AttnBlock (GroupNorm + single-head spatial attention + residual) on 8
Trainium2 NeuronCores.

Sharding: data-parallel over B (4 batches) x 2-way query-sequence parallel =
8 shards. Each core gets the full x[b] (rolled so its query half is the
first 2048 spatial positions), computes GroupNorm + Q/K/V projections +
attention for its 2048 queries + output projection + residual, and writes a
[512, 2048] slice of the output.

Compute layout (per core, C=512, S=4096, Sq=2048):
  x        [c, s]   4 chunks of [128, 4096] f32 (channels on partitions)
  h = GN(x)         4 chunks of [128, 4096] f16
  q = Wq h + bq     [128, 2048] f16 x4 (out-channels on partitions)
  k = Wk h + bk     [128, 4096] f16 x4
  vT = h^T Wv^T+bv  32 tiles of [128, 512] f16 (spatial on partitions!)
  scoresT[s,q] = k^T q   computed per (128-key-tile x 512-query-block) in
                 PSUM, exp()'d on ScalarE into SBUF f16 -- no transposes
                 anywhere: both AV operands already have s on partitions.
  out'[c,q] += vT^T e    accumulated over all 32 key tiles in 4 PSUM banks
  Z[q]     += 1^T e      (ones-matmul row)
  attn = out'/Z, proj = Wo attn + bo, out = x[:, :2048] + proj

GroupNorm stats use bn_stats/bn_aggr per channel + tiny indicator matmuls to
reduce/broadcast across the 16 channels of each group (cross-partition).

All heavy matmuls run in fp16 (1 PE cycle/row vs 4 for fp32); fp32 would be
~4x slower and fp16 end-to-end error is ~1e-4 of absmax (validated against
the fp32 reference).
"""
import numpy as np

import bass_rust
import concourse.bass as bass
import concourse.tile as tile
from concourse import mybir
from concourse.bass_utils import run_bass_kernel_spmd

F32 = mybir.dt.float32
F32R = mybir.dt.float32r
F16 = mybir.dt.float16
AF = mybir.ActivationFunctionType
ALU = mybir.AluOpType

B, C, H, W = 4, 512, 64, 64
S = H * W            # 4096 spatial positions (keys)
SQ = S // 2          # 2048 queries per core
CC = C // 128        # 4 channel chunks
ST = S // 128        # 32 key tiles
QB = SQ // 512       # 4 query blocks
NG = 32              # groups
GS = C // NG         # 16 channels per group
EPS = 1e-6
SCALE = 1.0 / float(np.sqrt(C))
VSCALE = 2.0 ** -6   # pre-scale on v and the Z-ones so |att| stays in f16


def _split_excess_waits(nc, max_waits=1):
    """walrus in this toolchain rejects instructions with >1 sync-wait.
    Hoist excess waits onto same-engine NOPs placed just before the
    instruction (engine streams are in-order, so this is equivalent)."""
    for f in nc.m.functions:
        for bb in f.blocks:
            out = []
            for inst in bb.instructions:
                si = inst.sync_info
                if si is not None and len(si.on_wait) > max_waits:
                    waits = list(si.on_wait)
                    plain = [w for w in waits if w.wait_reg is None]
                    special = [w for w in waits if w.wait_reg is not None]
                    n_keep = max(0, max_waits - len(special))
                    hoist = plain[: len(plain) - n_keep] if n_keep < len(plain) else []
                    keep = plain[len(hoist):] + special
                    if len(keep) > max_waits:
                        out.append(inst)
                        continue
                    for j, w in enumerate(hoist):
                        nop = mybir.InstNoOp(name=f"{inst.name}-wsplit{j}")
                        nop.engine = inst.engine
                        nop.sync_info = bass_rust.SyncInfo(on_wait=[w], on_update=[])
                        out.append(nop)
                    inst.sync_info = bass_rust.SyncInfo(
                        on_wait=keep, on_update=list(si.on_update))
                out.append(inst)
            bb.instructions = out


def _build(with_bv=True):
    nc = bass.Bass(trn_type="TRN2")

    x_d = nc.dram_tensor("x", [C, S], F32, kind="ExternalInput")
    w_d = {n: nc.dram_tensor(n, [C, C], F16, kind="ExternalInput")
           for n in ("wqT", "wkT", "wvT", "woT")}
    bq_d = nc.dram_tensor("bqc", [128, CC], F32, kind="ExternalInput")
    bk_d = nc.dram_tensor("bkc", [128, CC], F32, kind="ExternalInput")
    bo_d = nc.dram_tensor("boc", [128, CC], F32, kind="ExternalInput")
    bv_d = nc.dram_tensor("bv16", [1, C], F16, kind="ExternalInput")
    ga_d = nc.dram_tensor("gammac", [128, CC], F32, kind="ExternalInput")
    be_d = nc.dram_tensor("betac", [128, CC], F32, kind="ExternalInput")
    ind_d = nc.dram_tensor("ind", [128, CC, NG], F32, kind="ExternalInput")
    indT_d = nc.dram_tensor("indT", [NG, CC, 128], F32, kind="ExternalInput")
    out_d = nc.dram_tensor("out", [CC, 128, SQ], F32, kind="ExternalOutput")

    with tile.TileContext(nc) as tc:
        from contextlib import ExitStack
        with ExitStack() as stack:
            const = stack.enter_context(tc.tile_pool(name="const", bufs=1))
            work = stack.enter_context(tc.tile_pool(name="work", bufs=3))
            p_res = stack.enter_context(tc.tile_pool(name="p_res", bufs=1))
            p_h = stack.enter_context(tc.tile_pool(name="p_h", bufs=1))

            # ---- constants ----
            w_sb = {}
            for n in ("wqT", "wkT", "wvT", "woT"):
                t = const.tile([128, CC, C], F16, name=f"{n}_sb")
                nc.sync.dma_start(out=t[:], in_=w_d[n].rearrange(
                    "(c p) o -> p c o", p=128))
                w_sb[n] = t
            bq_sb = const.tile([128, CC], F32, name="bq_sb")
            nc.sync.dma_start(out=bq_sb[:], in_=bq_d[:, :])
            bk_sb = const.tile([128, CC], F32, name="bk_sb")
            nc.sync.dma_start(out=bk_sb[:], in_=bk_d[:, :])
            bo_sb = const.tile([128, CC], F32, name="bo_sb")
            nc.sync.dma_start(out=bo_sb[:], in_=bo_d[:, :])
            bv_sb = const.tile([1, C], F16, name="bv_sb")
            nc.sync.dma_start(out=bv_sb[:], in_=bv_d[:, :])
            ga_sb = const.tile([128, CC], F32, name="ga_sb")
            nc.sync.dma_start(out=ga_sb[:], in_=ga_d[:, :])
            be_sb = const.tile([128, CC], F32, name="be_sb")
            nc.sync.dma_start(out=be_sb[:], in_=be_d[:, :])
            ind_sb = const.tile([128, CC, NG], F32, name="ind_sb")
            nc.sync.dma_start(out=ind_sb[:], in_=ind_d[:, :, :])
            indT_sb = const.tile([NG, CC, 128], F32, name="indT_sb")
            nc.sync.dma_start(out=indT_sb[:], in_=indT_d[:, :, :])

            ones_r16 = const.tile([1, 128], F16, name="ones_r16")
            nc.vector.memset(ones_r16[:], 1.0)
            # full-width ones: the Z matmul keeps the PE's fast-weight-load
            # mode (a 1-col lhsT would break FWL and tax neighboring MMs),
            # and its PSUM output is Z broadcast across all 128 partitions.
            ones_sq16 = const.tile([128, 128], F16, name="ones_sq16")
            nc.vector.memset(ones_sq16[:], VSCALE)
            eps_sb = const.tile([NG, 1], F32, name="eps_sb")
            nc.vector.memset(eps_sb[:], EPS)

            h16 = p_h.tile([128, CC, S], F16, name="h16")
            xres = p_res.tile([128, CC, SQ], F32, name="xres")

            # =========== Phase 1: load x + GroupNorm ===========
            with tc.tile_pool(name="p_x", bufs=1) as p_x, \
                 tc.tile_pool(name="ps_gn", bufs=2, space="PSUM") as ps_gn:
                xc = p_x.tile([128, CC, S], F32, name="xc")
                for i in range(CC):
                    for hh in range(2):
                        cols = slice(hh * SQ, (hh + 1) * SQ)
                        nc.sync.dma_start(
                            out=xc[:, i, cols],
                            in_=x_d[i * 128:(i + 1) * 128, cols])

                # per-channel [sum, sumsq]: sum on DVE (reduce), sumsq on
                # ScalarE (Square + accum_out) -- the two run in parallel
                stats2 = []
                for i in range(CC):
                    s2 = work.tile([128, 2], F32, name="s2", tag="gn_s2", bufs=4)
                    nc.vector.tensor_reduce(out=s2[:, 0:1], in_=xc[:, i, :],
                                            axis=mybir.AxisListType.X,
                                            op=ALU.add)
                    sq = p_x.tile([128, S], F32, name="sq", tag="sq", bufs=1)
                    nc.scalar.activation(sq[:], xc[:, i, :], AF.Square,
                                         accum_out=s2[:, 1:2])
                    stats2.append(s2)

                # copy residual half out before x dies -- on the (otherwise
                # idle) DMA engines so neither DVE nor ACT pays for it
                for i in range(CC):
                    nc.sync.dma_start(out=xres[:, i, :], in_=xc[:, i, :SQ])

                # reduce over the 16 channels of each group: indicator matmul
                psg = ps_gn.tile([NG, 2], F32, name="psg")
                for i in range(CC):
                    nc.tensor.matmul(psg[:], ind_sb[:, i, :], stats2[i][:],
                                     start=(i == 0), stop=(i == CC - 1))
                gstat = work.tile([NG, 2], F32, name="gstat")
                nc.scalar.mul(gstat[:], psg[:], 1.0 / (GS * S))  # [mean_g, E_g[x^2]]

                # var_g, rstd_g (+ one Newton step to clean up ACT sqrt)
                mg2 = work.tile([NG, 1], F32, name="mg2")
                nc.vector.tensor_mul(mg2[:], gstat[:, 0:1], gstat[:, 0:1])
                varg = work.tile([NG, 1], F32, name="varg")
                nc.vector.tensor_tensor(out=varg[:], in0=gstat[:, 1:2],
                                        in1=mg2[:], op=ALU.subtract)
                sd = work.tile([NG, 1], F32, name="sd")
                nc.scalar.activation(sd[:], varg[:], AF.Sqrt, bias=eps_sb[:])
                r0 = work.tile([NG, 1], F32, name="r0")
                nc.vector.reciprocal(r0[:], sd[:])
                ve = work.tile([NG, 1], F32, name="ve")
                nc.vector.tensor_scalar_add(ve[:], varg[:], EPS)
                r0sq = work.tile([NG, 1], F32, name="r0sq")
                nc.vector.tensor_mul(r0sq[:], r0[:], r0[:])
                t2 = work.tile([NG, 1], F32, name="t2")
                nc.vector.tensor_mul(t2[:], ve[:], r0sq[:])
                t3 = work.tile([NG, 1], F32, name="t3")
                nc.vector.tensor_scalar(out=t3[:], in0=t2[:], scalar1=-0.5,
                                        scalar2=1.5, op0=ALU.mult, op1=ALU.add)
                gv = work.tile([NG, 2], F32, name="gv")  # [mean_g, rstd_g]
                nc.vector.tensor_copy(gv[:, 0:1], gstat[:, 0:1])
                nc.vector.tensor_mul(gv[:, 1:2], r0[:], t3[:])

                # broadcast group stats back to channels, fold gamma/beta
                for i in range(CC):
                    psb = ps_gn.tile([128, 2], F32, name="psb")
                    nc.tensor.matmul(psb[:], indT_sb[:, i, :], gv[:],
                                     start=True, stop=True)
                    mr = work.tile([128, 2], F32, name="mr", tag="gn_mr", bufs=4)
                    nc.scalar.copy(mr[:], psb[:])
                    sc_c = work.tile([128, 1], F32, name="sc_c", tag="gn_sc", bufs=4)
                    nc.vector.tensor_mul(sc_c[:], mr[:, 1:2], ga_sb[:, i:i + 1])
                    mt = work.tile([128, 1], F32, name="mt", tag="gn_mt", bufs=4)
                    nc.vector.tensor_mul(mt[:], mr[:, 0:1], sc_c[:])
                    bi_c = work.tile([128, 1], F32, name="bi_c", tag="gn_bi", bufs=4)
                    nc.vector.tensor_tensor(out=bi_c[:], in0=be_sb[:, i:i + 1],
                                            in1=mt[:], op=ALU.subtract)
                    # h = x*scale + bias, cast to f16 -- alternate the big
                    # [128, 4096] applies between ScalarE and VectorE
                    if i % 2 == 0:
                        nc.scalar.activation(h16[:, i, :], xc[:, i, :],
                                             AF.Identity, bias=bi_c[:],
                                             scale=sc_c[:])
                    else:
                        nc.vector.tensor_scalar(
                            out=h16[:, i, :], in0=xc[:, i, :],
                            scalar1=sc_c[:], scalar2=bi_c[:],
                            op0=ALU.mult, op1=ALU.add)

            # =========== Phase 2: projections ===========
            p_kv = stack.enter_context(tc.tile_pool(name="p_kv", bufs=1))
            k16 = p_kv.tile([128, CC, S], F16, name="k16")
            q16 = p_kv.tile([128, CC, SQ], F16, name="q16")
            vT16 = p_kv.tile([128, ST, C], F16, name="vT16")

            with tc.tile_pool(name="ps_proj", bufs=3, space="PSUM") as ps_p:
                # q = WqT^T h (+bq): only the first SQ columns of h
                for oc in range(CC):
                    for qb in range(SQ // 512):
                        pt = ps_p.tile([128, 512], F32, name="pt", tag="pp")
                        cols = slice(qb * 512, (qb + 1) * 512)
                        for ic in range(CC):
                            nc.tensor.matmul(
                                pt[:], w_sb["wqT"][:, ic, oc * 128:(oc + 1) * 128],
                                h16[:, ic, cols],
                                start=(ic == 0), stop=(ic == CC - 1))
                        nc.scalar.activation(q16[:, oc, cols], pt[:],
                                             AF.Identity, bias=bq_sb[:, oc:oc + 1])
                # k = WkT^T h (+bk): all S columns
                for oc in range(CC):
                    for sb in range(S // 512):
                        pt = ps_p.tile([128, 512], F32, name="pt", tag="pp")
                        cols = slice(sb * 512, (sb + 1) * 512)
                        for ic in range(CC):
                            nc.tensor.matmul(
                                pt[:], w_sb["wkT"][:, ic, oc * 128:(oc + 1) * 128],
                                h16[:, ic, cols],
                                start=(ic == 0), stop=(ic == CC - 1))
                        nc.scalar.activation(k16[:, oc, cols], pt[:],
                                             AF.Identity, bias=bk_sb[:, oc:oc + 1])
                # vT[s, c] = h[:, s]^T WvT (+bv broadcast via ones-matmul).
                # vT is stored pre-scaled by 2^-6 (and the Z-ones column uses
                # the same scale) so the unnormalized attention accumulator
                # stays comfortably inside f16 range; the scale cancels in
                # the final (Wo att)/Z normalization.
                for st in range(ST):
                    pt = ps_p.tile([128, 512], F32, name="pt", tag="pp")
                    scols = slice(st * 128, (st + 1) * 128)
                    for ic in range(CC):
                        nc.tensor.matmul(pt[:], h16[:, ic, scols],
                                         w_sb["wvT"][:, ic, :],
                                         start=(ic == 0),
                                         stop=(ic == CC - 1 and not with_bv))
                    if with_bv:
                        nc.tensor.matmul(pt[:], ones_r16[:], bv_sb[:],
                                         start=False, stop=True)
                    nc.scalar.mul(vT16[:, st, :], pt[:], VSCALE)

            # =========== Phase 3: attention + out-projection ===========
            # att (= 2^-6 * sum_s e[s,q] v[:,s], unnormalized) is evacuated
            # to f16 right after the key loop; normalization by 1/Z happens
            # AFTER the out-projection (it commutes with Wo), so the
            # reciprocal/broadcast chain runs on DVE off the PE critical
            # path. The out-projection for block qb is emitted after block
            # qb+1's key loop so its PSUM->f16 dependency is fully hidden.
            with tc.tile_pool(name="ps_po", bufs=4, space="PSUM") as ps_po, \
                 tc.tile_pool(name="ps_z", bufs=1, space="PSUM") as ps_z, \
                 tc.tile_pool(name="ps_s", bufs=3, space="PSUM") as ps_s:

                def emit_outproj(qb, att, rzb):
                    qcols = slice(qb * 512, (qb + 1) * 512)
                    for oc in range(CC):
                        pp = ps_s.tile([128, 512], F32, name="pp", tag="msum")
                        for cc2 in range(CC):
                            nc.tensor.matmul(
                                pp[:],
                                w_sb["woT"][:, cc2, oc * 128:(oc + 1) * 128],
                                att[cc2][:],
                                start=(cc2 == 0), stop=(cc2 == CC - 1))
                        t32 = work.tile([128, 512], F32, name="t32", tag="t32", bufs=2)
                        nc.vector.tensor_mul(t32[:], pp[:], rzb[:])
                        o32 = work.tile([128, 512], F32, name="o32", tag="o32", bufs=2)
                        nc.vector.scalar_tensor_tensor(
                            out=o32[:], in0=t32[:], scalar=bo_sb[:, oc:oc + 1],
                            in1=xres[:, oc, qcols], op0=ALU.add, op1=ALU.add)
                        nc.sync.dma_start(out=out_d[oc, :, qcols], in_=o32[:])

                def emit_scores(qb, st):
                    qcols = slice(qb * 512, (qb + 1) * 512)
                    pscore = ps_s.tile([128, 512], F32, name="pscore",
                                       tag="msum")
                    scols = slice(st * 128, (st + 1) * 128)
                    for ic in range(CC):
                        nc.tensor.matmul(pscore[:], k16[:, ic, scols],
                                         q16[:, ic, qcols],
                                         start=(ic == 0), stop=(ic == CC - 1))
                    e16 = work.tile([128, 512], F16, name="e16",
                                    tag="e16", bufs=4)
                    nc.scalar.activation(e16[:], pscore[:], AF.Exp,
                                         scale=SCALE)
                    return e16

                def emit_av(po, pz, st, e16):
                    for cc2 in range(CC):
                        nc.tensor.matmul(
                            po[cc2][:],
                            vT16[:, st, cc2 * 128:(cc2 + 1) * 128],
                            e16[:],
                            start=(st == 0), stop=(st == ST - 1))
                    nc.tensor.matmul(pz[:], ones_sq16[:], e16[:],
                                     start=(st == 0), stop=(st == ST - 1))

                prev = None
                for qb in range(QB):
                    po = [ps_po.tile([128, 512], F32, name="po", tag="po")
                          for _ in range(CC)]
                    pz = ps_z.tile([128, 512], F32, name="pz", tag="pz")
                    # software-pipelined: scores/exp for key-tile st+1 are
                    # issued before the AV matmuls of key-tile st, so the PE
                    # never waits on the ScalarE exp.
                    e_prev = emit_scores(qb, 0)
                    for st in range(1, ST):
                        e_cur = emit_scores(qb, st)
                        emit_av(po, pz, st - 1, e_prev)
                        e_prev = e_cur
                    emit_av(po, pz, ST - 1, e_prev)
                    # evacuate att to f16 (frees po for the next block);
                    # 1/Z on DVE off the PE critical path
                    att = [work.tile([128, 512], F16, name="att",
                                     tag="att", bufs=8) for _ in range(CC)]
                    for cc2 in range(CC):
                        nc.vector.tensor_copy(att[cc2][:], po[cc2][:])
                    zb = work.tile([128, 512], F32, name="zb", tag="zb", bufs=2)
                    nc.vector.tensor_copy(zb[:], pz[:])
                    # previous block's out-projection goes on DVE *before*
                    # this block's reciprocal so the PE's PSUM slots recycle
                    # without waiting on the 3us reciprocal
                    if prev is not None:
                        emit_outproj(*prev)
                    rzb = work.tile([128, 512], F32, name="rzb", tag="rzb",
                                    bufs=2)
                    nc.vector.reciprocal(rzb[:], zb[:])
                    prev = (qb, att, rzb)
                emit_outproj(*prev)

    _split_excess_waits(nc)
    return nc


_cache = {}


def _get_program(with_bv):
    key = ("nc", with_bv)
    if key not in _cache:
        _cache[key] = _build(with_bv)
    return _cache[key]


def kernel(x, gamma, beta, wq, bq, wk, bk, wv, bv, wo, bo, trace=False):
    x = np.asarray(x, dtype=np.float32)
    gamma = np.asarray(gamma, dtype=np.float32)
    beta = np.asarray(beta, dtype=np.float32)
    wq, wk, wv, wo = (np.asarray(a, dtype=np.float32) for a in (wq, wk, wv, wo))
    bq, bk, bv, bo = (np.asarray(a, dtype=np.float32) for a in (bq, bk, bv, bo))

    nc = _get_program(with_bv=bool(np.any(bv)))

    shared = {
        "wqT": np.ascontiguousarray(wq.T).astype(np.float16),
        "wkT": np.ascontiguousarray(wk.T).astype(np.float16),
        "wvT": np.ascontiguousarray(wv.T).astype(np.float16),
        "woT": np.ascontiguousarray(wo.T).astype(np.float16),
        "bqc": np.ascontiguousarray(bq.reshape(CC, 128).T),
        "bkc": np.ascontiguousarray(bk.reshape(CC, 128).T),
        "boc": np.ascontiguousarray(bo.reshape(CC, 128).T),
        "bv16": bv.reshape(1, C).astype(np.float16),
        "gammac": np.ascontiguousarray(gamma.reshape(CC, 128).T),
        "betac": np.ascontiguousarray(beta.reshape(CC, 128).T),
    }
    ind = np.zeros((128, CC, NG), np.float32)
    indT = np.zeros((NG, CC, 128), np.float32)
    for i in range(CC):
        for p in range(128):
            g = (i * 128 + p) // GS
            ind[p, i, g] = 1.0
            indT[g, i, p] = 1.0
    shared["ind"] = ind
    shared["indT"] = indT

    in_maps = []
    for core in range(8):
        b, half = core // 2, core % 2
        xs = x[b].reshape(C, S)
        if half:
            xin = np.concatenate([xs[:, SQ:], xs[:, :SQ]], axis=1)
        else:
            xin = np.ascontiguousarray(xs)
        in_maps.append({"x": xin, **shared})

    res = run_bass_kernel_spmd(nc, in_maps, core_ids=list(range(8)),
                               trace=trace)
    _cache["last_exec_time_ns"] = res.exec_time_ns

    y = np.empty((B, C, S), np.float32)
    for core in range(8):
        b, half = core // 2, core % 2
        y[b, :, half * SQ:(half + 1) * SQ] = \
            res.results[core]["out"].reshape(C, SQ)
    return y.reshape(B, C, H, W)


# revision 37
# speedup vs baseline: 1.1864x; 1.0001x over previous
"""AttnBlock (GroupNorm + single-head spatial attention + residual) on 8
Trainium2 NeuronCores.

Sharding: data-parallel over B (4 batches) x 2-way query-sequence parallel =
8 shards. Each core gets the full x[b] (rolled so its query half is the
first 2048 spatial positions), computes GroupNorm + Q/K/V projections +
attention for its 2048 queries + output projection + residual, and writes a
[512, 2048] slice of the output.

Compute layout (per core, C=512, S=4096, Sq=2048):
  x        [c, s]   4 chunks of [128, 4096] f32 (channels on partitions)
  h = GN(x)         4 chunks of [128, 4096] f16
  q = Wq h + bq     [128, 2048] f16 x4 (out-channels on partitions)
  k = Wk h + bk     [128, 4096] f16 x4
  vT = h^T Wv^T+bv  32 tiles of [128, 512] f16 (spatial on partitions!)
  scoresT[s,q] = k^T q   computed per (128-key-tile x 512-query-block) in
                 PSUM, exp()'d on ScalarE into SBUF f16 -- no transposes
                 anywhere: both AV operands already have s on partitions.
  out'[c,q] += vT^T e    accumulated over all 32 key tiles in 4 PSUM banks
  Z[q]     += 1^T e      (ones-matmul row)
  attn = out'/Z, proj = Wo attn + bo, out = x[:, :2048] + proj

GroupNorm stats use bn_stats/bn_aggr per channel + tiny indicator matmuls to
reduce/broadcast across the 16 channels of each group (cross-partition).

All heavy matmuls run in fp16 (1 PE cycle/row vs 4 for fp32); fp32 would be
~4x slower and fp16 end-to-end error is ~1e-4 of absmax (validated against
the fp32 reference).
"""
import numpy as np

import bass_rust
import concourse.bass as bass
import concourse.tile as tile
from concourse import mybir
from concourse.bass_utils import run_bass_kernel_spmd

F32 = mybir.dt.float32
F32R = mybir.dt.float32r
F16 = mybir.dt.float16
AF = mybir.ActivationFunctionType
ALU = mybir.AluOpType

B, C, H, W = 4, 512, 64, 64
S = H * W            # 4096 spatial positions (keys)
SQ = S // 2          # 2048 queries per core
CC = C // 128        # 4 channel chunks
ST = S // 128        # 32 key tiles
QB = SQ // 512       # 4 query blocks
NG = 32              # groups
GS = C // NG         # 16 channels per group
EPS = 1e-6
SCALE = 1.0 / float(np.sqrt(C))
VSCALE = 2.0 ** -6   # pre-scale on v and the Z-ones so |att| stays in f16


def _split_excess_waits(nc, max_waits=1):
    """walrus in this toolchain rejects instructions with >1 sync-wait.
    Hoist excess waits onto same-engine NOPs placed just before the
    instruction (engine streams are in-order, so this is equivalent)."""
    for f in nc.m.functions:
        for bb in f.blocks:
            out = []
            for inst in bb.instructions:
                si = inst.sync_info
                if si is not None and len(si.on_wait) > max_waits:
                    waits = list(si.on_wait)
                    plain = [w for w in waits if w.wait_reg is None]
                    special = [w for w in waits if w.wait_reg is not None]
                    n_keep = max(0, max_waits - len(special))
                    hoist = plain[: len(plain) - n_keep] if n_keep < len(plain) else []
                    keep = plain[len(hoist):] + special
                    if len(keep) > max_waits:
                        out.append(inst)
                        continue
                    for j, w in enumerate(hoist):
                        nop = mybir.InstNoOp(name=f"{inst.name}-wsplit{j}")
                        nop.engine = inst.engine
                        nop.sync_info = bass_rust.SyncInfo(on_wait=[w], on_update=[])
                        out.append(nop)
                    inst.sync_info = bass_rust.SyncInfo(
                        on_wait=keep, on_update=list(si.on_update))
                out.append(inst)
            bb.instructions = out


def _build(with_bv=True):
    nc = bass.Bass(trn_type="TRN2")

    x_d = nc.dram_tensor("x", [C, S], F32, kind="ExternalInput")
    w_d = {n: nc.dram_tensor(n, [C, C], F16, kind="ExternalInput")
           for n in ("wqT", "wkT", "wvT", "woT")}
    bq_d = nc.dram_tensor("bqc", [128, CC], F32, kind="ExternalInput")
    bk_d = nc.dram_tensor("bkc", [128, CC], F32, kind="ExternalInput")
    bo_d = nc.dram_tensor("boc", [128, CC], F32, kind="ExternalInput")
    bv_d = nc.dram_tensor("bv16", [1, C], F16, kind="ExternalInput")
    ga_d = nc.dram_tensor("gammac", [128, CC], F32, kind="ExternalInput")
    be_d = nc.dram_tensor("betac", [128, CC], F32, kind="ExternalInput")
    ind_d = nc.dram_tensor("ind", [128, CC, NG], F32, kind="ExternalInput")
    indT_d = nc.dram_tensor("indT", [NG, CC, 128], F32, kind="ExternalInput")
    out_d = nc.dram_tensor("out", [CC, 128, SQ], F32, kind="ExternalOutput")

    with tile.TileContext(nc) as tc:
        from contextlib import ExitStack
        with ExitStack() as stack:
            const = stack.enter_context(tc.tile_pool(name="const", bufs=1))
            work = stack.enter_context(tc.tile_pool(name="work", bufs=3))
            p_res = stack.enter_context(tc.tile_pool(name="p_res", bufs=1))
            p_h = stack.enter_context(tc.tile_pool(name="p_h", bufs=1))

            # ---- constants (weight DMAs are emitted after the x DMAs so
            # they queue behind x on the DGE queues, not ahead of it) ----
            w_sb = {}
            for n in ("wqT", "wkT", "wvT", "woT"):
                t = const.tile([128, CC, C], F16, name=f"{n}_sb")
                w_sb[n] = t

            def emit_weight_dmas():
                for j, n in enumerate(("wqT", "wkT", "wvT", "woT")):
                    eng = nc.sync if j % 2 == 0 else nc.gpsimd
                    eng.dma_start(out=w_sb[n][:], in_=w_d[n].rearrange(
                        "(c p) o -> p c o", p=128))

            bq_sb = const.tile([128, CC], F32, name="bq_sb")
            nc.sync.dma_start(out=bq_sb[:], in_=bq_d[:, :])
            bk_sb = const.tile([128, CC], F32, name="bk_sb")
            nc.sync.dma_start(out=bk_sb[:], in_=bk_d[:, :])
            bo_sb = const.tile([128, CC], F32, name="bo_sb")
            nc.sync.dma_start(out=bo_sb[:], in_=bo_d[:, :])
            bv_sb = const.tile([1, C], F16, name="bv_sb")
            nc.sync.dma_start(out=bv_sb[:], in_=bv_d[:, :])
            ga_sb = const.tile([128, CC], F32, name="ga_sb")
            nc.sync.dma_start(out=ga_sb[:], in_=ga_d[:, :])
            be_sb = const.tile([128, CC], F32, name="be_sb")
            nc.sync.dma_start(out=be_sb[:], in_=be_d[:, :])
            ind_sb = const.tile([128, CC, NG], F32, name="ind_sb")
            nc.sync.dma_start(out=ind_sb[:], in_=ind_d[:, :, :])
            indT_sb = const.tile([NG, CC, 128], F32, name="indT_sb")
            nc.sync.dma_start(out=indT_sb[:], in_=indT_d[:, :, :])

            ones_r16 = const.tile([1, 128], F16, name="ones_r16")
            nc.vector.memset(ones_r16[:], 1.0)
            # full-width ones: the Z matmul keeps the PE's fast-weight-load
            # mode (a 1-col lhsT would break FWL and tax neighboring MMs),
            # and its PSUM output is Z broadcast across all 128 partitions.
            ones_sq16 = const.tile([128, 128], F16, name="ones_sq16")
            nc.vector.memset(ones_sq16[:], VSCALE)
            eps_sb = const.tile([NG, 1], F32, name="eps_sb")
            nc.vector.memset(eps_sb[:], EPS)

            h16 = p_h.tile([128, CC, S], F16, name="h16")
            xres = p_res.tile([128, CC, SQ], F32, name="xres")

            # warm the ScalarE natural_log_exp table set while the input DMAs
            # are still in flight (the set load is ~2.7us and all ACT
            # functions used below -- Ln/Exp/Identity/Copy -- live in it)
            warm = work.tile([1, 2], F32, name="warm", tag="warm")
            nc.vector.memset(warm[:], 0.0)
            nc.scalar.activation(warm[:, 1:2], warm[:, 0:1], AF.Exp)

            # =========== Phase 1: load x + GroupNorm ===========
            with tc.tile_pool(name="p_x", bufs=1) as p_x, \
                 tc.tile_pool(name="ps_gn", bufs=2, space="PSUM") as ps_gn:
                xc = p_x.tile([128, CC, S], F32, name="xc")
                # 16 quarter-DMAs across both the HW and SW DGE queues so the
                # first chunk lands as early as possible (a single queue runs
                # at ~50 GB/s; the 16 together saturate HBM)
                for i in range(CC):
                    for qq in range(4):
                        cols = slice(qq * 1024, (qq + 1) * 1024)
                        eng = nc.sync if qq % 2 == 0 else nc.gpsimd
                        eng.dma_start(out=xc[:, i, cols],
                                      in_=x_d[i * 128:(i + 1) * 128, cols])
                emit_weight_dmas()

                # per-channel [sum, sumsq]: sumsq on DVE (tensor_tensor_reduce),
                # sum on ScalarE (Identity + accum_out) -- the two run in
                # parallel and neither needs an extra ACT table set
                stats2 = []
                for i in range(CC):
                    s2 = work.tile([128, 2], F32, name="s2", tag="gn_s2", bufs=4)
                    nc.vector.tensor_reduce(out=s2[:, 0:1], in_=xc[:, i, :],
                                            axis=mybir.AxisListType.X,
                                            op=ALU.add)
                    sq = p_x.tile([128, S], F16, name="sq", tag="sq", bufs=1)
                    nc.scalar.activation(sq[:], xc[:, i, :], AF.Square,
                                         accum_out=s2[:, 1:2])
                    stats2.append(s2)

                # copy residual half out before x dies -- on the (otherwise
                # idle) DMA engines so neither DVE nor ACT pays for it
                for i in range(CC):
                    nc.sync.dma_start(out=xres[:, i, :], in_=xc[:, i, :SQ])

                # reduce over the 16 channels of each group: indicator matmul
                psg = ps_gn.tile([NG, 2], F32, name="psg")
                for i in range(CC):
                    nc.tensor.matmul(psg[:], ind_sb[:, i, :], stats2[i][:],
                                     start=(i == 0), stop=(i == CC - 1))
                gstat = work.tile([NG, 2], F32, name="gstat")
                nc.scalar.mul(gstat[:], psg[:], 1.0 / (GS * S))  # [mean_g, E_g[x^2]]

                # rstd_g = (var+eps)^-0.5 via exp(-0.5*ln(var+eps)) -- Ln and
                # Exp share the already-loaded table set (Sqrt would force a
                # set switch) -- plus one Newton step for full fp32 accuracy
                nve = work.tile([NG, 1], F32, name="nve")  # mean^2 - E[x^2]
                nc.vector.scalar_tensor_tensor(
                    out=nve[:], in0=gstat[:, 0:1], scalar=gstat[:, 0:1],
                    in1=gstat[:, 1:2], op0=ALU.mult, op1=ALU.subtract)
                lnv = work.tile([NG, 1], F32, name="lnv")
                nc.scalar.activation(lnv[:], nve[:], AF.Ln, scale=-1.0,
                                     bias=eps_sb[:])
                r0 = work.tile([NG, 1], F32, name="r0")
                nc.scalar.activation(r0[:], lnv[:], AF.Exp, scale=-0.5)
                ve = work.tile([NG, 1], F32, name="ve")
                nc.scalar.activation(ve[:], nve[:], AF.Identity, scale=-1.0,
                                     bias=eps_sb[:])
                r0sq = work.tile([NG, 1], F32, name="r0sq")
                nc.vector.tensor_mul(r0sq[:], r0[:], r0[:])
                t2 = work.tile([NG, 1], F32, name="t2")
                nc.vector.tensor_mul(t2[:], ve[:], r0sq[:])
                t3 = work.tile([NG, 1], F32, name="t3")
                nc.vector.tensor_scalar(out=t3[:], in0=t2[:], scalar1=-0.5,
                                        scalar2=1.5, op0=ALU.mult, op1=ALU.add)
                gv = work.tile([NG, 2], F32, name="gv")  # [mean_g, rstd_g]
                nc.vector.tensor_copy(gv[:, 0:1], gstat[:, 0:1])
                nc.vector.tensor_mul(gv[:, 1:2], r0[:], t3[:])

                # broadcast group stats back to channels, fold gamma/beta
                for i in range(CC):
                    psb = ps_gn.tile([128, 2], F32, name="psb")
                    nc.tensor.matmul(psb[:], indT_sb[:, i, :], gv[:],
                                     start=True, stop=True)
                    mr = work.tile([128, 2], F32, name="mr", tag="gn_mr", bufs=4)
                    nc.scalar.copy(mr[:], psb[:])
                    sc_c = work.tile([128, 1], F32, name="sc_c", tag="gn_sc", bufs=4)
                    nc.vector.tensor_mul(sc_c[:], mr[:, 1:2], ga_sb[:, i:i + 1])
                    mt = work.tile([128, 1], F32, name="mt", tag="gn_mt", bufs=4)
                    nc.vector.tensor_mul(mt[:], mr[:, 0:1], sc_c[:])
                    bi_c = work.tile([128, 1], F32, name="bi_c", tag="gn_bi", bufs=4)
                    nc.vector.tensor_tensor(out=bi_c[:], in0=be_sb[:, i:i + 1],
                                            in1=mt[:], op=ALU.subtract)
                    # h = x*scale + bias, cast to f16 -- alternate the big
                    # [128, 4096] applies between ScalarE and VectorE
                    if i % 2 == 0:
                        nc.scalar.activation(h16[:, i, :], xc[:, i, :],
                                             AF.Identity, bias=bi_c[:],
                                             scale=sc_c[:])
                    else:
                        nc.vector.tensor_scalar(
                            out=h16[:, i, :], in0=xc[:, i, :],
                            scalar1=sc_c[:], scalar2=bi_c[:],
                            op0=ALU.mult, op1=ALU.add)

            # =========== Phase 2: projections ===========
            p_kv = stack.enter_context(tc.tile_pool(name="p_kv", bufs=1))
            k16 = p_kv.tile([128, CC, S], F16, name="k16")
            q16 = p_kv.tile([128, CC, SQ], F16, name="q16")
            vT16 = p_kv.tile([128, ST, C], F16, name="vT16")

            with tc.tile_pool(name="ps_proj", bufs=3, space="PSUM") as ps_p:
                # q = WqT^T h (+bq): only the first SQ columns of h
                for oc in range(CC):
                    for qb in range(SQ // 512):
                        pt = ps_p.tile([128, 512], F32, name="pt", tag="pp")
                        cols = slice(qb * 512, (qb + 1) * 512)
                        for ic in range(CC):
                            nc.tensor.matmul(
                                pt[:], w_sb["wqT"][:, ic, oc * 128:(oc + 1) * 128],
                                h16[:, ic, cols],
                                start=(ic == 0), stop=(ic == CC - 1))
                        nc.scalar.activation(q16[:, oc, cols], pt[:],
                                             AF.Identity, bias=bq_sb[:, oc:oc + 1])
                # k = WkT^T h (+bk): all S columns
                for oc in range(CC):
                    for sb in range(S // 512):
                        pt = ps_p.tile([128, 512], F32, name="pt", tag="pp")
                        cols = slice(sb * 512, (sb + 1) * 512)
                        for ic in range(CC):
                            nc.tensor.matmul(
                                pt[:], w_sb["wkT"][:, ic, oc * 128:(oc + 1) * 128],
                                h16[:, ic, cols],
                                start=(ic == 0), stop=(ic == CC - 1))
                        nc.scalar.activation(k16[:, oc, cols], pt[:],
                                             AF.Identity, bias=bk_sb[:, oc:oc + 1])
                # vT[s, c] = h[:, s]^T WvT (+bv broadcast via ones-matmul).
                # vT is stored pre-scaled by 2^-6 (and the Z-ones column uses
                # the same scale) so the unnormalized attention accumulator
                # stays comfortably inside f16 range; the scale cancels in
                # the final (Wo att)/Z normalization.
                for st in range(ST):
                    pt = ps_p.tile([128, 512], F32, name="pt", tag="pp")
                    scols = slice(st * 128, (st + 1) * 128)
                    for ic in range(CC):
                        nc.tensor.matmul(pt[:], h16[:, ic, scols],
                                         w_sb["wvT"][:, ic, :],
                                         start=(ic == 0),
                                         stop=(ic == CC - 1 and not with_bv))
                    if with_bv:
                        nc.tensor.matmul(pt[:], ones_r16[:], bv_sb[:],
                                         start=False, stop=True)
                    nc.scalar.mul(vT16[:, st, :], pt[:], VSCALE)

            # =========== Phase 3: attention + out-projection ===========
            # att (= 2^-6 * sum_s e[s,q] v[:,s], unnormalized) is evacuated
            # to f16 right after the key loop; normalization by 1/Z happens
            # AFTER the out-projection (it commutes with Wo), so the
            # reciprocal/broadcast chain runs on DVE off the PE critical
            # path. The out-projection for block qb is emitted after block
            # qb+1's key loop so its PSUM->f16 dependency is fully hidden.
            with tc.tile_pool(name="ps_po", bufs=4, space="PSUM") as ps_po, \
                 tc.tile_pool(name="ps_z", bufs=1, space="PSUM") as ps_z, \
                 tc.tile_pool(name="ps_s", bufs=3, space="PSUM") as ps_s:

                def emit_outproj(qb, att, rzb):
                    qcols = slice(qb * 512, (qb + 1) * 512)
                    for oc in range(CC):
                        pp = ps_s.tile([128, 512], F32, name="pp", tag="msum")
                        for cc2 in range(CC):
                            nc.tensor.matmul(
                                pp[:],
                                w_sb["woT"][:, cc2, oc * 128:(oc + 1) * 128],
                                att[cc2][:],
                                start=(cc2 == 0), stop=(cc2 == CC - 1))
                        t32 = work.tile([128, 512], F32, name="t32", tag="t32", bufs=2)
                        nc.vector.tensor_mul(t32[:], pp[:], rzb[:])
                        o32 = work.tile([128, 512], F32, name="o32", tag="o32", bufs=2)
                        nc.vector.scalar_tensor_tensor(
                            out=o32[:], in0=t32[:], scalar=bo_sb[:, oc:oc + 1],
                            in1=xres[:, oc, qcols], op0=ALU.add, op1=ALU.add)
                        nc.sync.dma_start(out=out_d[oc, :, qcols], in_=o32[:])

                def emit_scores(qb, st):
                    qcols = slice(qb * 512, (qb + 1) * 512)
                    pscore = ps_s.tile([128, 512], F32, name="pscore",
                                       tag="msum")
                    scols = slice(st * 128, (st + 1) * 128)
                    for ic in range(CC):
                        nc.tensor.matmul(pscore[:], k16[:, ic, scols],
                                         q16[:, ic, qcols],
                                         start=(ic == 0), stop=(ic == CC - 1))
                    e16 = work.tile([128, 512], F16, name="e16",
                                    tag="e16", bufs=4)
                    nc.scalar.activation(e16[:], pscore[:], AF.Exp,
                                         scale=SCALE)
                    return e16

                def emit_av(po, pz, st, e16):
                    for cc2 in range(CC):
                        nc.tensor.matmul(
                            po[cc2][:],
                            vT16[:, st, cc2 * 128:(cc2 + 1) * 128],
                            e16[:],
                            start=(st == 0), stop=(st == ST - 1))
                    nc.tensor.matmul(pz[:], ones_sq16[:], e16[:],
                                     start=(st == 0), stop=(st == ST - 1))

                prev = None
                for qb in range(QB):
                    po = [ps_po.tile([128, 512], F32, name="po", tag="po")
                          for _ in range(CC)]
                    pz = ps_z.tile([128, 512], F32, name="pz", tag="pz")
                    # software-pipelined: scores/exp for key-tile st+1 are
                    # issued before the AV matmuls of key-tile st, so the PE
                    # never waits on the ScalarE exp.
                    e_prev = emit_scores(qb, 0)
                    for st in range(1, ST):
                        e_cur = emit_scores(qb, st)
                        emit_av(po, pz, st - 1, e_prev)
                        e_prev = e_cur
                    emit_av(po, pz, ST - 1, e_prev)
                    # evacuate att to f16 (frees po for the next block);
                    # 1/Z on DVE off the PE critical path
                    att = [work.tile([128, 512], F16, name="att",
                                     tag="att", bufs=8) for _ in range(CC)]
                    for cc2 in range(CC):
                        nc.vector.tensor_copy(att[cc2][:], po[cc2][:])
                    zb = work.tile([128, 512], F32, name="zb", tag="zb", bufs=2)
                    nc.vector.tensor_copy(zb[:], pz[:])
                    # previous block's out-projection goes on DVE *before*
                    # this block's reciprocal so the PE's PSUM slots recycle
                    # without waiting on the 3us reciprocal; on the last block
                    # the reciprocal goes first to shorten the kernel tail
                    rzb = work.tile([128, 512], F32, name="rzb", tag="rzb",
                                    bufs=2)
                    if qb == QB - 1:
                        nc.vector.reciprocal(rzb[:], zb[:])
                        if prev is not None:
                            emit_outproj(*prev)
                    else:
                        if prev is not None:
                            emit_outproj(*prev)
                        nc.vector.reciprocal(rzb[:], zb[:])
                    prev = (qb, att, rzb)
                emit_outproj(*prev)

    _split_excess_waits(nc)
    return nc


_cache = {}


def _get_program(with_bv):
    key = ("nc", with_bv)
    if key not in _cache:
        _cache[key] = _build(with_bv)
    return _cache[key]


def kernel(x, gamma, beta, wq, bq, wk, bk, wv, bv, wo, bo, trace=False):
    x = np.asarray(x, dtype=np.float32)
    gamma = np.asarray(gamma, dtype=np.float32)
    beta = np.asarray(beta, dtype=np.float32)
    wq, wk, wv, wo = (np.asarray(a, dtype=np.float32) for a in (wq, wk, wv, wo))
    bq, bk, bv, bo = (np.asarray(a, dtype=np.float32) for a in (bq, bk, bv, bo))

    nc = _get_program(with_bv=bool(np.any(bv)))

    shared = {
        "wqT": np.ascontiguousarray(wq.T).astype(np.float16),
        "wkT": np.ascontiguousarray(wk.T).astype(np.float16),
        "wvT": np.ascontiguousarray(wv.T).astype(np.float16),
        "woT": np.ascontiguousarray(wo.T).astype(np.float16),
        "bqc": np.ascontiguousarray(bq.reshape(CC, 128).T),
        "bkc": np.ascontiguousarray(bk.reshape(CC, 128).T),
        "boc": np.ascontiguousarray(bo.reshape(CC, 128).T),
        "bv16": bv.reshape(1, C).astype(np.float16),
        "gammac": np.ascontiguousarray(gamma.reshape(CC, 128).T),
        "betac": np.ascontiguousarray(beta.reshape(CC, 128).T),
    }
    ind = np.zeros((128, CC, NG), np.float32)
    indT = np.zeros((NG, CC, 128), np.float32)
    for i in range(CC):
        for p in range(128):
            g = (i * 128 + p) // GS
            ind[p, i, g] = 1.0
            indT[g, i, p] = 1.0
    shared["ind"] = ind
    shared["indT"] = indT

    in_maps = []
    for core in range(8):
        b, half = core // 2, core % 2
        xs = x[b].reshape(C, S)
        if half:
            xin = np.concatenate([xs[:, SQ:], xs[:, :SQ]], axis=1)
        else:
            xin = np.ascontiguousarray(xs)
        in_maps.append({"x": xin, **shared})

    res = run_bass_kernel_spmd(nc, in_maps, core_ids=list(range(8)),
                               trace=trace)
    _cache["last_exec_time_ns"] = res.exec_time_ns

    y = np.empty((B, C, S), np.float32)
    for core in range(8):
        b, half = core // 2, core % 2
        y[b, :, half * SQ:(half + 1) * SQ] = \
            res.results[core]["out"].reshape(C, SQ)
    return y.reshape(B, C, H, W)
